# revision 80
# baseline (speedup 1.0000x reference)
"""Trainium2 Bass kernel for nn_BasicBlock1D (locally-connected 1x1 conv x2
with training-mode BatchNorm, residual, ReLU).

Reference computation (per spatial position h, there are H=64 of them):
    out1[n,o,h] = sum_c x[n,c,h] * w1[o,c,h]          (512x512 matmul per h)
    y1 = relu(bn1(out1))                              (stats over (N,H))
    out2[n,o,h] = sum_c y1[n,c,h] * w2[o,c,h]
    y  = relu(bn2(out2) + x)

Sharding: the 64 spatial positions are split across the 8 NeuronCores (8 per
core).  Each core reads only its h-slice of x/w1/w2, so every HBM byte is
read exactly once chip-wide.  BatchNorm statistics span the full (N,H) batch;
each core pre-aggregates its local per-channel (mean, E[x^2]) and a single
4KB AllReduce(add) with a Shared output buffer combines them:
    mean_g = sum_c mean_c / 8,  var_g = sum_c (var_c + mean_c^2)/8 - mean_g^2
(equal-sized groups of 2048 samples per core make this exact).

Schedule (cost-model-driven):
  The kernel is DMA-floor bound: 10MB of inputs + 2MB of output per core at
  the 360 B/ns aggregate DMA bandwidth is ~33us, against ~27us of bf16 PE
  time; the critical path is L1(DMA 6MB) -> bn1 chain -> L2(PE 13.7us) ->
  bn2 chain -> apply/store tail.

  * All inputs stream on the SP HWDGE ring in exact PE-consumption order at
    256KB granularity (x tiles in 2 halves, each 1MB weight tile in 4
    pieces), so layer 1 starts after ~1.1MB is buffered (~4.5us) and runs
    DMA-paced with the PE never stalling (a stalled PE drops out of its
    boosted clock: 394/213/107ns per 256-row matmul cold/mid/boosted, and
    any idle gap resets the boost).
  * g/b/eps constants ride as ONE batched DMA on the otherwise-idle ACT
    ring at t=0 (each DMA costs 625ns of the globally-serialized HWDGE
    sequencer regardless of size, so five separate loads would stall the
    main stream or arrive after the bn1 chain needs them).
  * PE warm-up fillers (garbage matmuls accumulating into a reserved PSUM
    bank) cover the initial prefetch window, and a second filler block is
    emitted contiguously behind layer 1's last matmul so the PE stays in
    its boosted clock state through the bn1-chain gap and layer 2 starts
    back-to-back at full speed.
  * Structure per h-pair: each PSUM tile is a full bank [128, 2, 256]; 8
    matmuls accumulate into it, a single ACT op evacuates it to bf16
    (walrus rejects GPSIMD touching PSUM at all), and bn_stats reads the
    PSUM bank directly so it runs concurrently with the evacuation.  The
    LAST h-pair's bn_stats are split into two half-bank tuples so the
    first half's stats overlap the second half's matmuls (bn_aggr
    combines tuples of unequal counts exactly).
  * The stats->coefficient chains are emitted PER output-channel chunk in
    the no-collective build (aggr -> (var+eps)^0.5 via a fused pow
    tensor_scalar -> g/std via tensor_tensor divide -> t), all on DVE so
    there are zero cross-engine handoffs; s[oc]/t[oc] unlock downstream
    per-oc work as soon as that oc's raw stats land.  The collective
    build keeps the proven monolithic Sqrt/reciprocal chain around the
    4KB AllReduce.
  * The final phase per tile: DVE affine (s2*o2 + t2, one fused 2-scalar
    tensor_scalar), residual add on DVE TT or Pool TT (walrus accepts
    ONLY the TensorTensor class on Pool -- no tensor_scalar, no max op),
    then relu as ACT Relu or a fused DVE (add 0, max) tensor_scalar for
    the end-tiles where ACT is congested.  Each output (oc, h-half)
    leaves in a 256KB store the moment its two tiles are applied.

A dummy Sqrt at t=0 preloads the ACT function table off the critical
path.

KERNEL_UNROLL=k (default 1) builds the whole pipeline k times back-to-back
into one NEFF -- a measurement aid.

Stack quirks this kernel deliberately avoids (verified empirically on this
axon/PJRT toolchain): tensor_tensor_reduce (faults), tensor_tensor with the
same tile as both operands, DVE memset feeding scalar operands, float
immediates in tensor_scalar, in-place elementwise ops, instructions whose
only output has no reader (walrus drops the alloc and the engine faults),
any non-TensorTensor compute op on Pool/GPSIMD and any Pool access to PSUM
(both fail walrus's per-engine ISA check), and SBUF tiles written by more
than one DMA (the framework under-gates readers of multi-DMA tiles, so
every DMA gets its own tile).
"""

import os
import sys
from contextlib import ExitStack

import numpy as np

_REPO = "/opt/trn_rl_repo"
if _REPO not in sys.path:
    sys.path.insert(0, _REPO)

import ml_dtypes  # noqa: E402

import concourse.bacc as bacc  # noqa: E402
import concourse.tile as tile  # noqa: E402
from concourse import mybir  # noqa: E402
from concourse.bass_utils import run_bass_kernel_spmd  # noqa: E402

N, C, H = 256, 512, 64
NCORES = 8
HS = H // NCORES  # 8 h positions per core
P = 128
KC = C // P  # 4 contraction chunks
OC = C // P  # 4 output-channel chunks
NN = N  # moving free dim of each matmul
HPAIRS = HS // 2  # weight tiles / activation tiles hold 2 h positions
EPS = 1e-5
GB_W = 4 * OC + 8  # packed g/b/cst width

BF16 = mybir.dt.bfloat16
F32 = mybir.dt.float32

LAST_EXEC_NS = None
LAST_RESULTS = None

_cached = None


def _build_program():
    nc = bacc.Bacc(
        "TRN2",
        target_bir_lowering=False,
        debug=False,
        num_devices=NCORES,
    )

    xt_d = nc.dram_tensor("xt", [HPAIRS, P, KC, 2, NN], BF16, kind="ExternalInput")
    w1_d = nc.dram_tensor("w1t", [HPAIRS, P, 2, KC, C], BF16, kind="ExternalInput")
    w2_d = nc.dram_tensor("w2t", [HPAIRS, P, 2, KC, C], BF16, kind="ExternalInput")
    gb_d = nc.dram_tensor("gbt", [P, GB_W], F32, kind="ExternalInput")
    unroll = int(os.environ.get("KERNEL_UNROLL", "1"))
    out_d = nc.dram_tensor(
        "out", [unroll, OC, P, HPAIRS * 2 * NN], BF16, kind="ExternalOutput"
    )
    junk_d = nc.dram_tensor("junk", [P, 2], F32, kind="ExternalOutput")

    add = mybir.AluOpType.add
    mult = mybir.AluOpType.mult
    amax = mybir.AluOpType.max
    AF = mybir.ActivationFunctionType
    use_cc = os.environ.get("KERNEL_NOCC", "0") != "1"

    NF1 = int(os.environ.get("KERNEL_NFILL1", "16"))
    NFM = int(os.environ.get("KERNEL_NFILLM", "20"))

    with tile.TileContext(nc) as tc, ExitStack() as ctx:
        persist = ctx.enter_context(tc.tile_pool(name="persist", bufs=1))
        wpool = ctx.enter_context(tc.tile_pool(name="wpool", bufs=32))
        spool = ctx.enter_context(tc.tile_pool(name="spool", bufs=2))
        psum = ctx.enter_context(tc.tile_pool(name="psum", bufs=7, space="PSUM"))
        psumf = ctx.enter_context(tc.tile_pool(name="psumf", bufs=1, space="PSUM"))
        fpool = ctx.enter_context(tc.tile_pool(name="fpool", bufs=1))
        dram = ctx.enter_context(tc.tile_pool(name="dram", bufs=1, space="DRAM"))

        def hp_tiles(nm, dt):
            return [
                [
                    persist.tile([P, 2, NN], dt, tag=f"{nm}_{k}_{hp}", name=f"{nm}_{k}_{hp}")
                    for hp in range(HPAIRS)
                ]
                for k in range(OC)
            ]

        # --- persistent activations (x in two kc-half tiles per h-pair so
        # the stream gates the first matmuls at 256KB granularity) ---
        xs = [
            [
                persist.tile([P, 2, 2, NN], BF16, tag=f"x{hp}_{kh}", name=f"x{hp}_{kh}")
                for kh in range(2)
            ]
            for hp in range(HPAIRS)
        ]

        def x_at(hp, kc, hh):
            return xs[hp][kc // 2][:, kc % 2, hh, :]
        raw1 = hp_tiles("r1", BF16)   # layer-1 pre-BN output
        y1 = hp_tiles("y1", BF16)
        o2 = hp_tiles("o2", BF16)     # layer-2 pre-BN output

        # g/b/cst: ONE batched DMA on the ACT ring at t=0 (see module doc).
        gbt = persist.tile([P, GB_W], F32, tag="gbt", name="gbt")
        nc.scalar.dma_start(out=gbt, in_=gb_d.ap())
        gbs = {
            "g1": gbt[:, 0 * OC : 1 * OC],
            "b1": gbt[:, 1 * OC : 2 * OC],
            "g2": gbt[:, 2 * OC : 3 * OC],
            "b2": gbt[:, 3 * OC : 4 * OC],
        }
        eps_ap = gbt[:, 4 * OC + 0 : 4 * OC + 1]
        inv8_ap = gbt[:, 4 * OC + 1 : 4 * OC + 2]  # 1/NCORES
        one_ap = gbt[:, 4 * OC + 2 : 4 * OC + 3]   # 1.0
        zero_ap = gbt[:, 4 * OC + 3 : 4 * OC + 4]  # 0.0
        half_ap = gbt[:, 4 * OC + 4 : 4 * OC + 5]  # 0.5

        # ACT function-table preload: a dummy Sqrt as the very first ACT op
        # pulls in the (sqrt + copy/relu/square) table off the critical path.
        # Its junk store is deferred to program end so the ACT SEQ doesn't
        # sit blocked on the HWDGE ring (busy with the input stream).
        dummy_in = persist.tile([P, 1], F32, tag="dmy_i", name="dmy_i")
        nc.vector.memset(dummy_in, 4.0)
        dummy_out = persist.tile([P, 1], F32, tag="dmy_o", name="dmy_o")
        nc.scalar.activation(out=dummy_out, in_=dummy_in, func=AF.Sqrt)

        # --- PE warm-up fillers (see module doc) ---
        filler_w = fpool.tile([P, P], BF16, tag="fil_w", name="fil_w")
        nc.gpsimd.memset(filler_w, 0.0)
        filler_x = fpool.tile([P, NN], BF16, tag="fil_x", name="fil_x")
        nc.gpsimd.memset(filler_x, 0.0)
        fpt = psumf.tile([P, NN], F32, tag="fil_p", name="fil_p")


        def pe_fill(n):
            # self-contained accumulation group per block: a group that
            # spans other groups' matmuls trips the scheduler into
            # serializing it behind unrelated work.  Returns (first, last)
            # instruction handles so callers can pin the block's position
            # in the PE stream with explicit deps (the tile scheduler
            # otherwise hoists dep-free matmuls into any earlier PE idle
            # slot).
            first = last = None
            for i in range(n):
                h = nc.tensor.matmul(
                    fpt, lhsT=filler_w, rhs=filler_x,
                    start=(i == 0), stop=(i == n - 1),
                )
                if first is None:
                    first = h
                last = h
            return first, last

        def w_piece(w_d, hp, hh, kcp):
            # a 256KB piece-TILE of a weight tile, in PE-consumption order:
            # (hh0,kc01) (hh0,kc23) (hh1,kc01) (hh1,kc23).  One DMA per
            # tile: the framework's DMA-completion waits under-gate tiles
            # written by multiple DMAs (verified in the timeline sim), so a
            # piece must be its own tile for readers to be sequenced right.
            wp = wpool.tile([P, 2, C], BF16, tag="wp", name="wp")
            nc.sync.dma_start(
                out=wp, in_=w_d.ap()[hp][:, hh, 2 * kcp : 2 * kcp + 2, :]
            )
            return wp

        def stream_l1(hp):
            # x halves interleaved with w pieces in first-need order
            nc.sync.dma_start(out=xs[hp][0], in_=xt_d.ap()[hp][:, 0:2])
            p00 = w_piece(w1_d, hp, 0, 0)
            p01 = w_piece(w1_d, hp, 0, 1)
            nc.sync.dma_start(out=xs[hp][1], in_=xt_d.ap()[hp][:, 2:4])
            p10 = w_piece(w1_d, hp, 1, 0)
            p11 = w_piece(w1_d, hp, 1, 1)
            return [p00, p01, p10, p11]

        def stream_w2(hp):
            return [w_piece(w2_d, hp, hh, kcp) for hh in range(2) for kcp in range(2)]

        def layer(w_tiles, src_at, dst_tiles, lname, tail_pairs=False):
            """Per-position matmuls + per-(chunk,h-pair) raw BN moments.

            src_at(kc, h) -> [P, NN] AP of the layer input
            dst_tiles[oc][hp][:, hh, :] <- the (h = 2*hp+hh) output slice
            tail_pairs: process the last two h-pairs interleaved by
            oc-PAIRS ((hp2,oc01),(hp3,oc01),(hp2,oc23),(hp3,oc23)) so
            oc0/oc1's raw stats complete ~2.6us before the layer ends and
            their coefficient chains + downstream applies/stores overlap
            the remaining matmuls.
            returns (stats tile [P, OC, HPAIRS, 6], first mm, last mm).
            """
            # HPAIRS+1 raw-stat slots per oc: one PSUM-direct tuple per
            # h-pair for hp<3, and TWO half-bank PSUM tuples for the last
            # h-pair so its first half's stats run while the second half's
            # matmuls are still going (bn_aggr combines tuples of unequal
            # counts exactly).
            st_raw = persist.tile(
                [P, OC, HPAIRS + 1, 6], F32, tag=f"straw_{lname}", name=f"straw_{lname}"
            )
            order = [(hp, oc) for hp in range(HPAIRS) for oc in range(OC)]
            if tail_pairs:
                order = [(hp, oc) for hp in (0, 1) for oc in range(OC)] + [
                    (hp, oc)
                    for ocp in (0, 1)
                    for hp in (2, 3)
                    for oc in (2 * ocp, 2 * ocp + 1)
                ]
            first_mm = last_mm = None
            for hp, oc in order:
                pieces = w_tiles[hp]
                if True:
                    # full-bank PSUM tile: both h of the pair
                    pt = psum.tile([P, 2, NN], F32, tag="ps", name="ps")
                    for hh in range(2):
                        h = hp * 2 + hh
                        for kc in range(KC):
                            wp = pieces[2 * hh + kc // 2]
                            m = nc.tensor.matmul(
                                pt[:, hh, :],
                                lhsT=wp[:, kc % 2, oc * P : (oc + 1) * P],
                                rhs=src_at(kc, h),
                                start=(kc == 0),
                                stop=(kc == KC - 1),
                            )
                            if first_mm is None:
                                first_mm = m
                            last_mm = m
                    # single-op PSUM evacuation on ACT (the only engine
                    # that can both read PSUM and run concurrently here:
                    # GPSIMD cannot access PSUM at all); bn_stats reads the
                    # PSUM bank directly so it runs concurrently with the
                    # evacuation instead of after it
                    nc.scalar.activation(
                        out=dst_tiles[oc][hp], in_=pt, func=AF.Copy
                    )
                    if hp == HPAIRS - 1:
                        for hh in range(2):
                            nc.vector.bn_stats(
                                out=st_raw[:, oc, hp + hh, :],
                                in_=pt[:, hh, :],
                            )
                    else:
                        nc.vector.bn_stats(
                            out=st_raw[:, oc, hp, :],
                            in_=pt.rearrange("p a n -> p (a n)"),
                        )
            return st_raw, first_mm, last_mm

        def stats_reduce(st_raw, g_t, b_t, lname, per_oc_done=None):
            """Local aggregate -> 4KB AllReduce(add) -> BN scale/shift.

            bn(v) = s*v + t with s = g/sqrt(var+eps), t = b - mean*s.
            """

            def small(nm, shape=(P, OC)):
                return persist.tile(
                    list(shape), F32, tag=f"{nm}_{lname}", name=f"{nm}_{lname}"
                )

            # Everything below stays on DVE (ACT only for the one Sqrt) --
            # each cross-engine handoff in this serial chain costs ~300ns.
            mv = small("mv", (P, OC, 2))
            if not use_cc:
                # per-oc chains: s[oc]/t[oc] unlock as soon as THAT oc's
                # last raw stats land, so downstream per-oc consumers
                # (y1 h-pair 0, the final apply) start ~1-2us earlier than
                # a monolithic chain would allow.
                std = small("std")
                s_t = small("s")
                mts = small("mts")
                t_t = small("t")
                for oc in range(OC):
                    sl = slice(oc, oc + 1)
                    nc.vector.bn_aggr(out=mv[:, oc, :], in_=st_raw[:, oc, :, :])
                    nc.vector.tensor_scalar(
                        out=std[:, sl], in0=mv[:, oc, 1:2], scalar1=eps_ap,
                        scalar2=half_ap, op0=add, op1=mybir.AluOpType.pow,
                    )
                    nc.vector.tensor_tensor(
                        out=s_t[:, sl], in0=g_t[:, sl], in1=std[:, sl],
                        op=mybir.AluOpType.divide,
                    )
                    nc.vector.tensor_tensor(
                        out=mts[:, sl], in0=mv[:, oc, 0:1], in1=s_t[:, sl],
                        op=mult,
                    )
                    nc.vector.tensor_tensor(
                        out=t_t[:, sl], in0=b_t[:, sl], in1=mts[:, sl],
                        op=mybir.AluOpType.subtract,
                    )
                    if per_oc_done is not None:
                        per_oc_done(oc, s_t, t_t)
                return s_t, t_t
            for oc in range(OC):
                nc.vector.bn_aggr(out=mv[:, oc, :], in_=st_raw[:, oc, :, :])
            if True:
                # pack local (mean, var + mean^2) pairs
                cin = small("cin", (P, OC, 2))
                nc.vector.tensor_scalar(
                    out=cin[:, :, 0], in0=mv[:, :, 0], scalar1=one_ap, scalar2=None,
                    op0=mult,
                )
                msq = small("msq")
                nc.vector.tensor_mul(out=msq, in0=mv[:, :, 0], in1=cin[:, :, 0])
                nc.vector.tensor_add(out=cin[:, :, 1], in0=mv[:, :, 1], in1=msq)
                cci = dram.tile([P, 2 * OC], F32, tag=f"cci_{lname}", name=f"cci_{lname}")
                cco = dram.tile(
                    [P, 2 * OC], F32, tag=f"cco_{lname}", name=f"cco_{lname}",
                    addr_space="Shared",
                )
                nc.scalar.dma_start(out=cci, in_=cin.rearrange("p a b -> p (a b)"))
                nc.gpsimd.collective_compute(
                    "AllReduce",
                    add,
                    replica_groups=[list(range(NCORES))],
                    ins=[cci.opt()],
                    outs=[cco.opt()],
                )
                red = small("red", (P, OC, 2))
                nc.scalar.dma_start(
                    out=red, in_=cco.rearrange("p (a b) -> p a b", a=OC)
                )
                me2 = small("me2", (P, OC, 2))
                nc.vector.tensor_scalar(
                    out=me2, in0=red, scalar1=inv8_ap, scalar2=None, op0=mult
                )
                m_g = me2[:, :, 0]
                mr = small("mr")
                nc.vector.tensor_mul(out=mr, in0=m_g, in1=red[:, :, 0])  # m^2 * 8
                varg = small("varg")  # var * 8
                nc.vector.tensor_sub(out=varg, in0=red[:, :, 1], in1=mr)
                std = small("std")
                nc.scalar.activation(
                    out=std, in_=varg, func=AF.Sqrt, bias=eps_ap, scale=inv8_ap
                )
                rstd = small("rstd")
                nc.vector.reciprocal(out=rstd, in_=std)
                s_t = small("s")
                nc.vector.tensor_mul(out=s_t, in0=rstd, in1=g_t)
            mts = small("mts")
            nc.vector.tensor_mul(out=mts, in0=m_g, in1=s_t)
            t_t = small("t")
            nc.vector.tensor_sub(out=t_t, in0=b_t, in1=mts)
            if per_oc_done is not None:
                for oc in range(OC):
                    per_oc_done(oc, s_t, t_t)
            return s_t, t_t

        from concourse.tile_rust import add_dep_helper

        for _u in range(unroll):
            _, f1_last = pe_fill(NF1)
            # ---- input stream: exact PE-consumption order on the SP ring
            w1_tiles = [stream_l1(hp) for hp in range(HPAIRS)]
            w2_tiles = [stream_w2(hp) for hp in range(HPAIRS)]

            # ---------------- layer 1 ----------------
            stats1, l1_first, l1_last = layer(
                w1_tiles, lambda kc, h: x_at(h // 2, kc, h % 2), raw1, f"l1_{_u}",
            )
            # keep the PE boosted through the bn1-chain gap: fillers run
            # back-to-back behind layer 1's last matmul, and layer 2's
            # first matmul queues back-to-back behind the fillers.  The
            # tile scheduler hoists dep-free matmuls into any earlier PE
            # idle slot, so the block is pinned with explicit deps.
            if f1_last is not None:
                add_dep_helper(l1_first.ins, f1_last.ins, reason="NF1 before L1")
            fm_first, fm_last = pe_fill(NFM)
            if fm_first is not None:
                add_dep_helper(fm_first.ins, l1_last.ins, reason="mid fillers after L1")
            # y1 = relu(s1*out1 + t1).  The h-pair-0 tile for each oc is
            # emitted from the per-oc chain hook so the first layer-2
            # matmul group unblocks as early as possible.  Engine split
            # balances hp0 latency (ACT 1-op + DVE 2-op) and mid-L2
            # occupancy (ACT/DVE/Pool); Pool tiles go on late h-pairs
            # since its 2-op path is ~3.2us for 2 tiles.
            Y1_ACT = {(2, 0), (3, 0), (0, 1), (1, 1), (0, 2), (1, 2), (0, 3)}

            def y1_tile(oc, hp, s1, t1):
                if (oc, hp) in Y1_ACT:
                    nc.scalar.activation(
                        out=y1[oc][hp],
                        in_=raw1[oc][hp],
                        func=AF.Relu,
                        scale=s1[:, oc : oc + 1],
                        bias=t1[:, oc : oc + 1],
                    )
                else:
                    ytmp = spool.tile([P, 2, NN], BF16, tag="ya", name="ya", bufs=3)
                    nc.vector.tensor_scalar(
                        out=ytmp,
                        in0=raw1[oc][hp],
                        scalar1=s1[:, oc : oc + 1],
                        scalar2=t1[:, oc : oc + 1],
                        op0=mult,
                        op1=add,
                    )
                    nc.vector.tensor_scalar(
                        out=y1[oc][hp], in0=ytmp, scalar1=zero_ap,
                        scalar2=None, op0=amax,
                    )

            hook1 = (
                (lambda oc, s, t: y1_tile(oc, 0, s, t))
                if os.environ.get("KERNEL_Y1_HOOK", "1") == "1"
                else None
            )
            s1, t1 = stats_reduce(
                stats1, gbs["g1"], gbs["b1"], f"l1_{_u}", per_oc_done=hook1
            )
            for hp in range(0 if hook1 is None else 1, HPAIRS):
                for oc in range(OC):
                    y1_tile(oc, hp, s1, t1)

            # ---------------- layer 2 ----------------
            stats2, l2_first, l2_last = layer(
                w2_tiles, lambda kc, h: y1[kc][h // 2][:, h % 2, :], o2, f"l2_{_u}",
            )
            if fm_last is not None:
                add_dep_helper(l2_first.ins, fm_last.ins, reason="L2 after mid fillers")
            # y = relu(s2*out2 + t2 + x): DVE affine TSP (194), then the
            # residual add on DVE TT (327) or Pool TT (1111, PATH_C), then
            # relu on ACT (612) or DVE fused (add,max) TSP (194, PATH_A).
            # Stores leave per (oc, half) as soon as the half is applied.
            PATH_C = {(0, 1), (1, 1), (2, 1), (3, 1), (0, 2), (1, 2)}
            PATH_A = {(2, 2), (3, 2), (0, 3), (1, 3), (2, 3), (3, 3)}
            outbigs = [
                persist.tile(
                    [P, HPAIRS, 2, NN], BF16, tag=f"obig_{oc}", name=f"obig_{oc}"
                )
                for oc in range(OC)
            ]

            def apply_tile(oc, hp, s2, t2):
                outbig = outbigs[oc]
                s2c = s2[:, oc : oc + 1]
                t2c = t2[:, oc : oc + 1]
                x_in = xs[hp][oc // 2][:, oc % 2, :, :]
                aff = spool.tile([P, 2, NN], BF16, tag="f1", name="f1", bufs=8)
                nc.vector.tensor_scalar(
                    out=aff, in0=o2[oc][hp], scalar1=s2c, scalar2=t2c,
                    op0=mult, op1=add,
                )
                v = spool.tile([P, 2, NN], BF16, tag="f2", name="f2", bufs=8)
                e_add = nc.gpsimd if (oc, hp) in PATH_C else nc.vector
                e_add.tensor_tensor(out=v, in0=aff, in1=x_in, op=add)
                if (oc, hp) in PATH_A:
                    nc.vector.tensor_scalar(
                        out=outbig[:, hp, :, :], in0=v, scalar1=zero_ap,
                        scalar2=None, op0=amax,
                    )
                else:
                    nc.scalar.activation(
                        out=outbig[:, hp, :, :], in_=v, func=AF.Relu
                    )

            half = 2 * NN

            def apply_oc(oc, s2, t2):
                for hf in range(2):
                    apply_tile(oc, 2 * hf, s2, t2)
                    apply_tile(oc, 2 * hf + 1, s2, t2)
                    nc.sync.dma_start(
                        out=out_d.ap()[_u, oc][:, 2 * hf * half : (2 * hf + 2) * half],
                        in_=outbigs[oc][:, 2 * hf : 2 * hf + 2, :, :].rearrange(
                            "p a b n -> p (a b n)"
                        ),
                    )

            if os.environ.get("KERNEL_APPLY_HOOK", "1") == "1":
                stats_reduce(
                    stats2, gbs["g2"], gbs["b2"], f"l2_{_u}", per_oc_done=apply_oc
                )
            else:
                s2, t2 = stats_reduce(stats2, gbs["g2"], gbs["b2"], f"l2_{_u}")
                for hf in range(2):
                    for oc in range(OC):
                        apply_tile(oc, 2 * hf, s2, t2)
                        apply_tile(oc, 2 * hf + 1, s2, t2)
                        nc.sync.dma_start(
                            out=out_d.ap()[_u, oc][
                                :, 2 * hf * half : (2 * hf + 2) * half
                            ],
                            in_=outbigs[oc][:, 2 * hf : 2 * hf + 2, :, :].rearrange(
                                "p a b n -> p (a b n)"
                            ),
                        )

        nc.scalar.dma_start(out=junk_d.ap()[:, 0:1], in_=dummy_out)
        if NF1 + NFM > 0:
            jnk2 = persist.tile([P, 1], F32, tag="jnk2", name="jnk2")
            nc.scalar.activation(out=jnk2, in_=fpt[:, 0:1], func=AF.Copy)
            nc.scalar.dma_start(out=junk_d.ap()[:, 1:2], in_=jnk2)

    nc.compile()
    return nc


def _get_program():
    global _cached
    if _cached is None:
        _cached = _build_program()
    return _cached


def _pack_inputs(x, w1, g1, b1, w2, g2, b2):
    """Host-side shard + repack into the device layouts (see module doc)."""
    bf16 = ml_dtypes.bfloat16
    # x: (N, C, H) -> (KC, P, H, N) -> per-core (HPAIRS, P, KC, 2, N)
    xt = np.ascontiguousarray(x.transpose(1, 2, 0)).reshape(KC, P, H, N)
    xt = xt.astype(bf16)

    # w: (O, C, H) -> (H, P, KC, C)
    def packw(w):
        wt = w.transpose(2, 1, 0).reshape(H, KC, P, C).transpose(0, 2, 1, 3)
        return wt.astype(bf16)

    w1t = packw(w1)
    w2t = packw(w2)

    def packg(v):
        return np.ascontiguousarray(v.reshape(OC, P).T.astype(np.float32))

    gbt = np.empty((P, GB_W), np.float32)
    gbt[:, 0 * OC : 1 * OC] = packg(g1)
    gbt[:, 1 * OC : 2 * OC] = packg(b1)
    gbt[:, 2 * OC : 3 * OC] = packg(g2)
    gbt[:, 3 * OC : 4 * OC] = packg(b2)
    gbt[:, 4 * OC + 0] = EPS
    gbt[:, 4 * OC + 1] = 1.0 / NCORES
    gbt[:, 4 * OC + 2] = 1.0
    gbt[:, 4 * OC + 3] = 0.0
    gbt[:, 4 * OC + 4] = 0.5
    gbt[:, 4 * OC + 5 :] = 0.0

    in_maps = []
    for c in range(NCORES):
        h0, h1 = c * HS, (c + 1) * HS
        # (KC, P, HS, N) -> (KC, P, HPAIRS, 2, N) -> (HPAIRS, P, KC, 2, N)
        xc = xt[:, :, h0:h1, :].reshape(KC, P, HPAIRS, 2, N)
        xc = np.ascontiguousarray(xc.transpose(2, 1, 0, 3, 4))
        in_maps.append(
            {
                "xt": xc,
                "w1t": np.ascontiguousarray(w1t[h0:h1]).reshape(
                    HPAIRS, 2, P, KC, C
                ).transpose(0, 2, 1, 3, 4).copy(),
                "w2t": np.ascontiguousarray(w2t[h0:h1]).reshape(
                    HPAIRS, 2, P, KC, C
                ).transpose(0, 2, 1, 3, 4).copy(),
                "gbt": gbt,
            }
        )
    return in_maps


def kernel(x, w1, g1, b1, w2, g2, b2):
    global LAST_EXEC_NS, LAST_RESULTS
    nc = _get_program()
    in_maps = _pack_inputs(
        np.asarray(x, dtype=np.float32),
        np.asarray(w1, dtype=np.float32),
        np.asarray(g1, dtype=np.float32),
        np.asarray(b1, dtype=np.float32),
        np.asarray(w2, dtype=np.float32),
        np.asarray(g2, dtype=np.float32),
        np.asarray(b2, dtype=np.float32),
    )
    trace = os.environ.get("KERNEL_TRACE", "0") == "1"
    res = run_bass_kernel_spmd(
        nc, in_maps, list(range(NCORES)), trace=trace
    )
    LAST_EXEC_NS = res.exec_time_ns
    LAST_RESULTS = res
    parts = []
    for c in range(NCORES):
        r = np.asarray(res.results[c]["out"])[0]  # [oc, p, (hp hh n)] bf16
        r = r.astype(np.float32).reshape(OC, P, HPAIRS, 2, N)
        # -> (n, c, h): c = oc*P + p, h = 2*hp + hh
        r = r.transpose(4, 0, 1, 2, 3).reshape(N, C, HS)
        parts.append(r)
    return np.concatenate(parts, axis=2).astype(np.float32)


if __name__ == "__main__":
    # smoke test with random data
    rng = np.random.default_rng(0)
    x = rng.standard_normal((N, C, H), dtype=np.float32)
    w1 = rng.standard_normal((C, C, H), dtype=np.float32) * 0.02
    w2 = rng.standard_normal((C, C, H), dtype=np.float32) * 0.02
    g1 = np.ones(C, np.float32)
    b1 = np.zeros(C, np.float32)
    g2 = np.ones(C, np.float32)
    b2 = np.zeros(C, np.float32)
    y = kernel(x=x, w1=w1, g1=g1, b1=b1, w2=w2, g2=g2, b2=b2)
    print(y.shape, y.dtype, float(np.abs(y).max()))


# revision 84
# speedup vs baseline: 1.0134x; 1.0134x over previous
"""Trainium2 Bass kernel for nn_BasicBlock1D (locally-connected 1x1 conv x2
with training-mode BatchNorm, residual, ReLU).

Reference computation (per spatial position h, there are H=64 of them):
    out1[n,o,h] = sum_c x[n,c,h] * w1[o,c,h]          (512x512 matmul per h)
    y1 = relu(bn1(out1))                              (stats over (N,H))
    out2[n,o,h] = sum_c y1[n,c,h] * w2[o,c,h]
    y  = relu(bn2(out2) + x)

Sharding: the 64 spatial positions are split across the 8 NeuronCores (8 per
core).  Each core reads only its h-slice of x/w1/w2, so every HBM byte is
read exactly once chip-wide.  BatchNorm statistics span the full (N,H) batch;
each core pre-aggregates its local per-channel (mean, E[x^2]) and a single
4KB AllReduce(add) with a Shared output buffer combines them:
    mean_g = sum_c mean_c / 8,  var_g = sum_c (var_c + mean_c^2)/8 - mean_g^2
(equal-sized groups of 2048 samples per core make this exact).

Schedule (cost-model-driven):
  The kernel is DMA-floor bound: 10MB of inputs + 2MB of output per core at
  the 360 B/ns aggregate DMA bandwidth is ~33us, against ~27us of bf16 PE
  time; the critical path is L1(DMA 6MB) -> bn1 chain -> L2(PE 13.7us) ->
  bn2 chain -> apply/store tail.

  * All inputs stream on the SP HWDGE ring in exact PE-consumption order at
    256KB granularity (x tiles in 2 halves, each 1MB weight tile in 4
    pieces), so layer 1 starts after ~1.1MB is buffered (~4.5us) and runs
    DMA-paced with the PE never stalling (a stalled PE drops out of its
    boosted clock: 394/213/107ns per 256-row matmul cold/mid/boosted, and
    any idle gap resets the boost).
  * g/b/eps constants ride as ONE batched DMA on the otherwise-idle ACT
    ring at t=0 (each DMA costs 625ns of the globally-serialized HWDGE
    sequencer regardless of size, so five separate loads would stall the
    main stream or arrive after the bn1 chain needs them).
  * PE warm-up fillers (garbage matmuls accumulating into a reserved PSUM
    bank) cover the initial prefetch window, and a second filler block is
    emitted contiguously behind layer 1's last matmul so the PE stays in
    its boosted clock state through the bn1-chain gap and layer 2 starts
    back-to-back at full speed.
  * Structure per h-pair: each PSUM tile is a full bank [128, 2, 256]; 8
    matmuls accumulate into it, a single ACT op evacuates it to bf16
    (walrus rejects GPSIMD touching PSUM at all), and bn_stats reads the
    PSUM bank directly so it runs concurrently with the evacuation.  The
    LAST h-pair's bn_stats are split into two half-bank tuples so the
    first half's stats overlap the second half's matmuls (bn_aggr
    combines tuples of unequal counts exactly).
  * The stats->coefficient chains are emitted PER output-channel chunk in
    the no-collective build (aggr -> (var+eps)^0.5 via a fused pow
    tensor_scalar -> g/std via tensor_tensor divide -> t), all on DVE so
    there are zero cross-engine handoffs; s[oc]/t[oc] unlock downstream
    per-oc work as soon as that oc's raw stats land.  The collective
    build keeps the proven monolithic Sqrt/reciprocal chain around the
    4KB AllReduce.
  * The final phase per tile: DVE affine (s2*o2 + t2, one fused 2-scalar
    tensor_scalar), residual add on DVE TT or Pool TT (walrus accepts
    ONLY the TensorTensor class on Pool -- no tensor_scalar, no max op),
    then relu as ACT Relu or a fused DVE (add 0, max) tensor_scalar for
    the end-tiles where ACT is congested.  Each output (oc, h-half)
    leaves in a 256KB store the moment its two tiles are applied.

A dummy Sqrt at t=0 preloads the ACT function table off the critical
path.

KERNEL_UNROLL=k (default 1) builds the whole pipeline k times back-to-back
into one NEFF -- a measurement aid.

Stack quirks this kernel deliberately avoids (verified empirically on this
axon/PJRT toolchain): tensor_tensor_reduce (faults), tensor_tensor with the
same tile as both operands, DVE memset feeding scalar operands, float
immediates in tensor_scalar, in-place elementwise ops, instructions whose
only output has no reader (walrus drops the alloc and the engine faults),
any non-TensorTensor compute op on Pool/GPSIMD and any Pool access to PSUM
(both fail walrus's per-engine ISA check), and SBUF tiles written by more
than one DMA (the framework under-gates readers of multi-DMA tiles, so
every DMA gets its own tile).
"""

import os
import sys
from contextlib import ExitStack

import numpy as np

_REPO = "/opt/trn_rl_repo"
if _REPO not in sys.path:
    sys.path.insert(0, _REPO)

import ml_dtypes  # noqa: E402

import concourse.bacc as bacc  # noqa: E402
import concourse.tile as tile  # noqa: E402
from concourse import mybir  # noqa: E402
from concourse.bass_utils import run_bass_kernel_spmd  # noqa: E402

N, C, H = 256, 512, 64
NCORES = 8
HS = H // NCORES  # 8 h positions per core
P = 128
KC = C // P  # 4 contraction chunks
OC = C // P  # 4 output-channel chunks
NN = N  # moving free dim of each matmul
HPAIRS = HS // 2  # weight tiles / activation tiles hold 2 h positions
EPS = 1e-5
GB_W = 4 * OC + 8  # packed g/b/cst width

BF16 = mybir.dt.bfloat16
F32 = mybir.dt.float32

LAST_EXEC_NS = None
LAST_RESULTS = None

_cached = None


def _build_program():
    nc = bacc.Bacc(
        "TRN2",
        target_bir_lowering=False,
        debug=False,
        num_devices=NCORES,
    )

    xt_d = nc.dram_tensor("xt", [HPAIRS, P, KC, 2, NN], BF16, kind="ExternalInput")
    w1_d = nc.dram_tensor("w1t", [HPAIRS, P, 2, KC, C], BF16, kind="ExternalInput")
    w2_d = nc.dram_tensor("w2t", [HPAIRS, P, 2, KC, C], BF16, kind="ExternalInput")
    gb_d = nc.dram_tensor("gbt", [P, GB_W], F32, kind="ExternalInput")
    unroll = int(os.environ.get("KERNEL_UNROLL", "1"))
    out_d = nc.dram_tensor(
        "out", [unroll, OC, P, HPAIRS * 2 * NN], BF16, kind="ExternalOutput"
    )
    junk_d = nc.dram_tensor("junk", [P, 2], F32, kind="ExternalOutput")

    add = mybir.AluOpType.add
    mult = mybir.AluOpType.mult
    amax = mybir.AluOpType.max
    AF = mybir.ActivationFunctionType
    use_cc = os.environ.get("KERNEL_NOCC", "0") != "1"

    NF1 = int(os.environ.get("KERNEL_NFILL1", "16"))
    NFM = int(os.environ.get("KERNEL_NFILLM", "20"))

    with tile.TileContext(nc) as tc, ExitStack() as ctx:
        persist = ctx.enter_context(tc.tile_pool(name="persist", bufs=1))
        wpool = ctx.enter_context(tc.tile_pool(name="wpool", bufs=32))
        spool = ctx.enter_context(tc.tile_pool(name="spool", bufs=2))
        psum = ctx.enter_context(tc.tile_pool(name="psum", bufs=7, space="PSUM"))
        psumf = ctx.enter_context(tc.tile_pool(name="psumf", bufs=1, space="PSUM"))
        fpool = ctx.enter_context(tc.tile_pool(name="fpool", bufs=1))
        dram = ctx.enter_context(tc.tile_pool(name="dram", bufs=1, space="DRAM"))

        def hp_tiles(nm, dt):
            return [
                [
                    persist.tile([P, 2, NN], dt, tag=f"{nm}_{k}_{hp}", name=f"{nm}_{k}_{hp}")
                    for hp in range(HPAIRS)
                ]
                for k in range(OC)
            ]

        # --- persistent activations (x in two kc-half tiles per h-pair so
        # the stream gates the first matmuls at 256KB granularity) ---
        xs = [
            [
                persist.tile([P, 2, 2, NN], BF16, tag=f"x{hp}_{kh}", name=f"x{hp}_{kh}")
                for kh in range(2)
            ]
            for hp in range(HPAIRS)
        ]

        def x_at(hp, kc, hh):
            return xs[hp][kc // 2][:, kc % 2, hh, :]
        raw1 = hp_tiles("r1", BF16)   # layer-1 pre-BN output
        y1 = hp_tiles("y1", BF16)
        o2 = hp_tiles("o2", BF16)     # layer-2 pre-BN output

        # g/b/cst: ONE batched DMA on the ACT ring at t=0 (see module doc).
        gbt = persist.tile([P, GB_W], F32, tag="gbt", name="gbt")
        nc.scalar.dma_start(out=gbt, in_=gb_d.ap())
        gbs = {
            "g1": gbt[:, 0 * OC : 1 * OC],
            "b1": gbt[:, 1 * OC : 2 * OC],
            "g2": gbt[:, 2 * OC : 3 * OC],
            "b2": gbt[:, 3 * OC : 4 * OC],
        }
        eps_ap = gbt[:, 4 * OC + 0 : 4 * OC + 1]
        inv8_ap = gbt[:, 4 * OC + 1 : 4 * OC + 2]  # 1/NCORES
        one_ap = gbt[:, 4 * OC + 2 : 4 * OC + 3]   # 1.0
        zero_ap = gbt[:, 4 * OC + 3 : 4 * OC + 4]  # 0.0
        half_ap = gbt[:, 4 * OC + 4 : 4 * OC + 5]  # 0.5

        # ACT function-table preload: a dummy Sqrt as the very first ACT op
        # pulls in the (sqrt + copy/relu/square) table off the critical path.
        # Its junk store is deferred to program end so the ACT SEQ doesn't
        # sit blocked on the HWDGE ring (busy with the input stream).
        dummy_in = persist.tile([P, 1], F32, tag="dmy_i", name="dmy_i")
        nc.vector.memset(dummy_in, 4.0)
        dummy_out = persist.tile([P, 1], F32, tag="dmy_o", name="dmy_o")
        nc.scalar.activation(out=dummy_out, in_=dummy_in, func=AF.Sqrt)

        # --- PE warm-up fillers (see module doc) ---
        filler_w = fpool.tile([P, P], BF16, tag="fil_w", name="fil_w")
        nc.gpsimd.memset(filler_w, 0.0)
        filler_x = fpool.tile([P, NN], BF16, tag="fil_x", name="fil_x")
        nc.gpsimd.memset(filler_x, 0.0)
        fpt = psumf.tile([P, NN], F32, tag="fil_p", name="fil_p")


        def pe_fill(n):
            # self-contained accumulation group per block: a group that
            # spans other groups' matmuls trips the scheduler into
            # serializing it behind unrelated work.  Returns (first, last)
            # instruction handles so callers can pin the block's position
            # in the PE stream with explicit deps (the tile scheduler
            # otherwise hoists dep-free matmuls into any earlier PE idle
            # slot).
            first = last = None
            for i in range(n):
                h = nc.tensor.matmul(
                    fpt, lhsT=filler_w, rhs=filler_x,
                    start=(i == 0), stop=(i == n - 1),
                )
                if first is None:
                    first = h
                last = h
            return first, last

        def w_piece(w_d, hp, hh, kcp):
            # a 256KB piece-TILE of a weight tile, in PE-consumption order:
            # (hh0,kc01) (hh0,kc23) (hh1,kc01) (hh1,kc23).  One DMA per
            # tile: the framework's DMA-completion waits under-gate tiles
            # written by multiple DMAs (verified in the timeline sim), so a
            # piece must be its own tile for readers to be sequenced right.
            wp = wpool.tile([P, 2, C], BF16, tag="wp", name="wp")
            nc.sync.dma_start(
                out=wp, in_=w_d.ap()[hp][:, hh, 2 * kcp : 2 * kcp + 2, :]
            )
            return wp

        def stream_l1(hp):
            # x halves interleaved with w pieces in first-need order
            nc.sync.dma_start(out=xs[hp][0], in_=xt_d.ap()[hp][:, 0:2])
            p00 = w_piece(w1_d, hp, 0, 0)
            p01 = w_piece(w1_d, hp, 0, 1)
            nc.sync.dma_start(out=xs[hp][1], in_=xt_d.ap()[hp][:, 2:4])
            p10 = w_piece(w1_d, hp, 1, 0)
            p11 = w_piece(w1_d, hp, 1, 1)
            return [p00, p01, p10, p11]

        def stream_w2(hp):
            return [w_piece(w2_d, hp, hh, kcp) for hh in range(2) for kcp in range(2)]

        def layer(w_tiles, src_at, dst_tiles, lname, tail_pairs=False):
            """Per-position matmuls + per-(chunk,h-pair) raw BN moments.

            src_at(kc, h) -> [P, NN] AP of the layer input
            dst_tiles[oc][hp][:, hh, :] <- the (h = 2*hp+hh) output slice
            tail_pairs: process the last two h-pairs interleaved by
            oc-PAIRS ((hp2,oc01),(hp3,oc01),(hp2,oc23),(hp3,oc23)) so
            oc0/oc1's raw stats complete ~2.6us before the layer ends and
            their coefficient chains + downstream applies/stores overlap
            the remaining matmuls.
            returns (stats tile [P, OC, HPAIRS, 6], first mm, last mm).
            """
            # HPAIRS+1 raw-stat slots per oc: one PSUM-direct tuple per
            # h-pair for hp<3, and TWO half-bank PSUM tuples for the last
            # h-pair so its first half's stats run while the second half's
            # matmuls are still going (bn_aggr combines tuples of unequal
            # counts exactly).
            st_raw = persist.tile(
                [P, OC, HPAIRS + 1, 6], F32, tag=f"straw_{lname}", name=f"straw_{lname}"
            )
            order = [(hp, oc) for hp in range(HPAIRS) for oc in range(OC)]
            if tail_pairs:
                order = [(hp, oc) for hp in (0, 1) for oc in range(OC)] + [
                    (hp, oc)
                    for ocp in (0, 1)
                    for hp in (2, 3)
                    for oc in (2 * ocp, 2 * ocp + 1)
                ]
            first_mm = last_mm = None
            for hp in range(HPAIRS):
                pieces = w_tiles[hp]
                # hh-MAJOR matmul emission within the chunk: the first 16
                # matmuls need only the (hh0) weight pieces, so the PE
                # consumes the stream as it arrives instead of the first
                # oc-group stalling on the chunk's last piece.  Each
                # pt[:, hh, :] slice is its own accumulation group either
                # way.  For the last h-pair this also completes all hh0
                # half-bank stats inputs 1.7us before the layer ends.
                pts = [None] * OC
                for hh in range(2):
                    h = hp * 2 + hh
                    for oc in range(OC):
                        if pts[oc] is None:
                            # allocate at first use so the pool-rotation
                            # waits stagger instead of bunching at the
                            # chunk boundary
                            pts[oc] = psum.tile([P, 2, NN], F32, tag="ps", name="ps")
                        for kc in range(KC):
                            wp = pieces[2 * hh + kc // 2]
                            m = nc.tensor.matmul(
                                pts[oc][:, hh, :],
                                lhsT=wp[:, kc % 2, oc * P : (oc + 1) * P],
                                rhs=src_at(kc, h),
                                start=(kc == 0),
                                stop=(kc == KC - 1),
                            )
                            if first_mm is None:
                                first_mm = m
                            last_mm = m
                    if hp == HPAIRS - 1:
                        # half-bank stats as soon as this hh's groups close
                        for oc in range(OC):
                            nc.vector.bn_stats(
                                out=st_raw[:, oc, hp + hh, :],
                                in_=pts[oc][:, hh, :],
                            )
                for oc in range(OC):
                    # single-op PSUM evacuation on ACT (the only engine
                    # that can both read PSUM and run concurrently here:
                    # GPSIMD cannot access PSUM at all); bn_stats reads the
                    # PSUM bank directly so it runs concurrently with the
                    # evacuation instead of after it
                    nc.scalar.activation(
                        out=dst_tiles[oc][hp], in_=pts[oc], func=AF.Copy
                    )
                    if hp < HPAIRS - 1:
                        nc.vector.bn_stats(
                            out=st_raw[:, oc, hp, :],
                            in_=pts[oc].rearrange("p a n -> p (a n)"),
                        )
            return st_raw, first_mm, last_mm

        def stats_reduce(st_raw, g_t, b_t, lname, per_oc_done=None):
            """Local aggregate -> 4KB AllReduce(add) -> BN scale/shift.

            bn(v) = s*v + t with s = g/sqrt(var+eps), t = b - mean*s.
            """

            def small(nm, shape=(P, OC)):
                return persist.tile(
                    list(shape), F32, tag=f"{nm}_{lname}", name=f"{nm}_{lname}"
                )

            # Everything below stays on DVE (ACT only for the one Sqrt) --
            # each cross-engine handoff in this serial chain costs ~300ns.
            mv = small("mv", (P, OC, 2))
            if not use_cc:
                # per-oc chains: s[oc]/t[oc] unlock as soon as THAT oc's
                # last raw stats land, so downstream per-oc consumers
                # (y1 h-pair 0, the final apply) start ~1-2us earlier than
                # a monolithic chain would allow.
                std = small("std")
                s_t = small("s")
                mts = small("mts")
                t_t = small("t")
                for oc in range(OC):
                    sl = slice(oc, oc + 1)
                    nc.vector.bn_aggr(out=mv[:, oc, :], in_=st_raw[:, oc, :, :])
                    nc.vector.tensor_scalar(
                        out=std[:, sl], in0=mv[:, oc, 1:2], scalar1=eps_ap,
                        scalar2=half_ap, op0=add, op1=mybir.AluOpType.pow,
                    )
                    nc.vector.tensor_tensor(
                        out=s_t[:, sl], in0=g_t[:, sl], in1=std[:, sl],
                        op=mybir.AluOpType.divide,
                    )
                    nc.vector.tensor_tensor(
                        out=mts[:, sl], in0=mv[:, oc, 0:1], in1=s_t[:, sl],
                        op=mult,
                    )
                    nc.vector.tensor_tensor(
                        out=t_t[:, sl], in0=b_t[:, sl], in1=mts[:, sl],
                        op=mybir.AluOpType.subtract,
                    )
                    if per_oc_done is not None:
                        per_oc_done(oc, s_t, t_t)
                return s_t, t_t
            for oc in range(OC):
                nc.vector.bn_aggr(out=mv[:, oc, :], in_=st_raw[:, oc, :, :])
            if True:
                # pack local (mean, var + mean^2) pairs
                cin = small("cin", (P, OC, 2))
                nc.vector.tensor_scalar(
                    out=cin[:, :, 0], in0=mv[:, :, 0], scalar1=one_ap, scalar2=None,
                    op0=mult,
                )
                msq = small("msq")
                nc.vector.tensor_mul(out=msq, in0=mv[:, :, 0], in1=cin[:, :, 0])
                nc.vector.tensor_add(out=cin[:, :, 1], in0=mv[:, :, 1], in1=msq)
                cci = dram.tile([P, 2 * OC], F32, tag=f"cci_{lname}", name=f"cci_{lname}")
                cco = dram.tile(
                    [P, 2 * OC], F32, tag=f"cco_{lname}", name=f"cco_{lname}",
                    addr_space="Shared",
                )
                nc.scalar.dma_start(out=cci, in_=cin.rearrange("p a b -> p (a b)"))
                nc.gpsimd.collective_compute(
                    "AllReduce",
                    add,
                    replica_groups=[list(range(NCORES))],
                    ins=[cci.opt()],
                    outs=[cco.opt()],
                )
                red = small("red", (P, OC, 2))
                nc.scalar.dma_start(
                    out=red, in_=cco.rearrange("p (a b) -> p a b", a=OC)
                )
                me2 = small("me2", (P, OC, 2))
                nc.vector.tensor_scalar(
                    out=me2, in0=red, scalar1=inv8_ap, scalar2=None, op0=mult
                )
                m_g = me2[:, :, 0]
                mr = small("mr")
                nc.vector.tensor_mul(out=mr, in0=m_g, in1=red[:, :, 0])  # m^2 * 8
                varg = small("varg")  # var * 8
                nc.vector.tensor_sub(out=varg, in0=red[:, :, 1], in1=mr)
                std = small("std")
                nc.scalar.activation(
                    out=std, in_=varg, func=AF.Sqrt, bias=eps_ap, scale=inv8_ap
                )
                rstd = small("rstd")
                nc.vector.reciprocal(out=rstd, in_=std)
                s_t = small("s")
                nc.vector.tensor_mul(out=s_t, in0=rstd, in1=g_t)
            mts = small("mts")
            nc.vector.tensor_mul(out=mts, in0=m_g, in1=s_t)
            t_t = small("t")
            nc.vector.tensor_sub(out=t_t, in0=b_t, in1=mts)
            if per_oc_done is not None:
                for oc in range(OC):
                    per_oc_done(oc, s_t, t_t)
            return s_t, t_t

        from concourse.tile_rust import add_dep_helper

        for _u in range(unroll):
            _, f1_last = pe_fill(NF1)
            # ---- input stream: exact PE-consumption order on the SP ring
            w1_tiles = [stream_l1(hp) for hp in range(HPAIRS)]
            w2_tiles = [stream_w2(hp) for hp in range(HPAIRS)]

            # ---------------- layer 1 ----------------
            stats1, l1_first, l1_last = layer(
                w1_tiles, lambda kc, h: x_at(h // 2, kc, h % 2), raw1, f"l1_{_u}",
            )
            # keep the PE boosted through the bn1-chain gap: fillers run
            # back-to-back behind layer 1's last matmul, and layer 2's
            # first matmul queues back-to-back behind the fillers.  The
            # tile scheduler hoists dep-free matmuls into any earlier PE
            # idle slot, so the block is pinned with explicit deps.
            if f1_last is not None:
                add_dep_helper(l1_first.ins, f1_last.ins, reason="NF1 before L1")
            fm_first, fm_last = pe_fill(NFM)
            if fm_first is not None:
                add_dep_helper(fm_first.ins, l1_last.ins, reason="mid fillers after L1")
            # y1 = relu(s1*out1 + t1).  The h-pair-0 tile for each oc is
            # emitted from the per-oc chain hook so the first layer-2
            # matmul group unblocks as early as possible.  Engine split
            # balances hp0 latency (ACT 1-op + DVE 2-op) and mid-L2
            # occupancy (ACT/DVE/Pool); Pool tiles go on late h-pairs
            # since its 2-op path is ~3.2us for 2 tiles.
            Y1_ACT = {(2, 0), (3, 0), (0, 1), (1, 1), (0, 2), (1, 2), (0, 3)}

            def y1_tile(oc, hp, s1, t1):
                if (oc, hp) in Y1_ACT:
                    nc.scalar.activation(
                        out=y1[oc][hp],
                        in_=raw1[oc][hp],
                        func=AF.Relu,
                        scale=s1[:, oc : oc + 1],
                        bias=t1[:, oc : oc + 1],
                    )
                else:
                    ytmp = spool.tile([P, 2, NN], BF16, tag="ya", name="ya", bufs=3)
                    nc.vector.tensor_scalar(
                        out=ytmp,
                        in0=raw1[oc][hp],
                        scalar1=s1[:, oc : oc + 1],
                        scalar2=t1[:, oc : oc + 1],
                        op0=mult,
                        op1=add,
                    )
                    nc.vector.tensor_scalar(
                        out=y1[oc][hp], in0=ytmp, scalar1=zero_ap,
                        scalar2=None, op0=amax,
                    )

            hook1 = (
                (lambda oc, s, t: y1_tile(oc, 0, s, t))
                if os.environ.get("KERNEL_Y1_HOOK", "0") == "1"
                else None
            )
            s1, t1 = stats_reduce(
                stats1, gbs["g1"], gbs["b1"], f"l1_{_u}", per_oc_done=hook1
            )
            for hp in range(0 if hook1 is None else 1, HPAIRS):
                for oc in range(OC):
                    y1_tile(oc, hp, s1, t1)

            # ---------------- layer 2 ----------------
            stats2, l2_first, l2_last = layer(
                w2_tiles, lambda kc, h: y1[kc][h // 2][:, h % 2, :], o2, f"l2_{_u}",
            )
            if fm_last is not None:
                add_dep_helper(l2_first.ins, fm_last.ins, reason="L2 after mid fillers")
            # y = relu(s2*out2 + t2 + x): DVE affine TSP (194), then the
            # residual add on DVE TT (327) or Pool TT (1111, PATH_C), then
            # relu on ACT (612) or DVE fused (add,max) TSP (194, PATH_A).
            # Stores leave per (oc, half) as soon as the half is applied.
            PATH_C = {(0, 1), (1, 1), (2, 1), (3, 1), (0, 2), (1, 2)}
            PATH_A = {(2, 2), (3, 2), (0, 3), (1, 3), (2, 3), (3, 3)}
            outbigs = [
                persist.tile(
                    [P, HPAIRS, 2, NN], BF16, tag=f"obig_{oc}", name=f"obig_{oc}"
                )
                for oc in range(OC)
            ]

            def apply_tile(oc, hp, s2, t2):
                outbig = outbigs[oc]
                s2c = s2[:, oc : oc + 1]
                t2c = t2[:, oc : oc + 1]
                x_in = xs[hp][oc // 2][:, oc % 2, :, :]
                aff = spool.tile([P, 2, NN], BF16, tag="f1", name="f1", bufs=8)
                nc.vector.tensor_scalar(
                    out=aff, in0=o2[oc][hp], scalar1=s2c, scalar2=t2c,
                    op0=mult, op1=add,
                )
                v = spool.tile([P, 2, NN], BF16, tag="f2", name="f2", bufs=8)
                e_add = nc.gpsimd if (oc, hp) in PATH_C else nc.vector
                e_add.tensor_tensor(out=v, in0=aff, in1=x_in, op=add)
                if (oc, hp) in PATH_A:
                    nc.vector.tensor_scalar(
                        out=outbig[:, hp, :, :], in0=v, scalar1=zero_ap,
                        scalar2=None, op0=amax,
                    )
                else:
                    nc.scalar.activation(
                        out=outbig[:, hp, :, :], in_=v, func=AF.Relu
                    )

            half = 2 * NN

            def apply_oc(oc, s2, t2):
                for hf in range(2):
                    apply_tile(oc, 2 * hf, s2, t2)
                    apply_tile(oc, 2 * hf + 1, s2, t2)
                    nc.sync.dma_start(
                        out=out_d.ap()[_u, oc][:, 2 * hf * half : (2 * hf + 2) * half],
                        in_=outbigs[oc][:, 2 * hf : 2 * hf + 2, :, :].rearrange(
                            "p a b n -> p (a b n)"
                        ),
                    )

            if os.environ.get("KERNEL_APPLY_HOOK", "1") == "1":
                stats_reduce(
                    stats2, gbs["g2"], gbs["b2"], f"l2_{_u}", per_oc_done=apply_oc
                )
            else:
                s2, t2 = stats_reduce(stats2, gbs["g2"], gbs["b2"], f"l2_{_u}")
                for hf in range(2):
                    for oc in range(OC):
                        apply_tile(oc, 2 * hf, s2, t2)
                        apply_tile(oc, 2 * hf + 1, s2, t2)
                        nc.sync.dma_start(
                            out=out_d.ap()[_u, oc][
                                :, 2 * hf * half : (2 * hf + 2) * half
                            ],
                            in_=outbigs[oc][:, 2 * hf : 2 * hf + 2, :, :].rearrange(
                                "p a b n -> p (a b n)"
                            ),
                        )

        nc.scalar.dma_start(out=junk_d.ap()[:, 0:1], in_=dummy_out)
        if NF1 + NFM > 0:
            jnk2 = persist.tile([P, 1], F32, tag="jnk2", name="jnk2")
            nc.scalar.activation(out=jnk2, in_=fpt[:, 0:1], func=AF.Copy)
            nc.scalar.dma_start(out=junk_d.ap()[:, 1:2], in_=jnk2)

    nc.compile()
    return nc


def _get_program():
    global _cached
    if _cached is None:
        _cached = _build_program()
    return _cached


def _pack_inputs(x, w1, g1, b1, w2, g2, b2):
    """Host-side shard + repack into the device layouts (see module doc)."""
    bf16 = ml_dtypes.bfloat16
    # x: (N, C, H) -> (KC, P, H, N) -> per-core (HPAIRS, P, KC, 2, N)
    xt = np.ascontiguousarray(x.transpose(1, 2, 0)).reshape(KC, P, H, N)
    xt = xt.astype(bf16)

    # w: (O, C, H) -> (H, P, KC, C)
    def packw(w):
        wt = w.transpose(2, 1, 0).reshape(H, KC, P, C).transpose(0, 2, 1, 3)
        return wt.astype(bf16)

    w1t = packw(w1)
    w2t = packw(w2)

    def packg(v):
        return np.ascontiguousarray(v.reshape(OC, P).T.astype(np.float32))

    gbt = np.empty((P, GB_W), np.float32)
    gbt[:, 0 * OC : 1 * OC] = packg(g1)
    gbt[:, 1 * OC : 2 * OC] = packg(b1)
    gbt[:, 2 * OC : 3 * OC] = packg(g2)
    gbt[:, 3 * OC : 4 * OC] = packg(b2)
    gbt[:, 4 * OC + 0] = EPS
    gbt[:, 4 * OC + 1] = 1.0 / NCORES
    gbt[:, 4 * OC + 2] = 1.0
    gbt[:, 4 * OC + 3] = 0.0
    gbt[:, 4 * OC + 4] = 0.5
    gbt[:, 4 * OC + 5 :] = 0.0

    in_maps = []
    for c in range(NCORES):
        h0, h1 = c * HS, (c + 1) * HS
        # (KC, P, HS, N) -> (KC, P, HPAIRS, 2, N) -> (HPAIRS, P, KC, 2, N)
        xc = xt[:, :, h0:h1, :].reshape(KC, P, HPAIRS, 2, N)
        xc = np.ascontiguousarray(xc.transpose(2, 1, 0, 3, 4))
        in_maps.append(
            {
                "xt": xc,
                "w1t": np.ascontiguousarray(w1t[h0:h1]).reshape(
                    HPAIRS, 2, P, KC, C
                ).transpose(0, 2, 1, 3, 4).copy(),
                "w2t": np.ascontiguousarray(w2t[h0:h1]).reshape(
                    HPAIRS, 2, P, KC, C
                ).transpose(0, 2, 1, 3, 4).copy(),
                "gbt": gbt,
            }
        )
    return in_maps


def kernel(x, w1, g1, b1, w2, g2, b2):
    global LAST_EXEC_NS, LAST_RESULTS
    nc = _get_program()
    in_maps = _pack_inputs(
        np.asarray(x, dtype=np.float32),
        np.asarray(w1, dtype=np.float32),
        np.asarray(g1, dtype=np.float32),
        np.asarray(b1, dtype=np.float32),
        np.asarray(w2, dtype=np.float32),
        np.asarray(g2, dtype=np.float32),
        np.asarray(b2, dtype=np.float32),
    )
    trace = os.environ.get("KERNEL_TRACE", "0") == "1"
    res = run_bass_kernel_spmd(
        nc, in_maps, list(range(NCORES)), trace=trace
    )
    LAST_EXEC_NS = res.exec_time_ns
    LAST_RESULTS = res
    parts = []
    for c in range(NCORES):
        r = np.asarray(res.results[c]["out"])[0]  # [oc, p, (hp hh n)] bf16
        r = r.astype(np.float32).reshape(OC, P, HPAIRS, 2, N)
        # -> (n, c, h): c = oc*P + p, h = 2*hp + hh
        r = r.transpose(4, 0, 1, 2, 3).reshape(N, C, HS)
        parts.append(r)
    return np.concatenate(parts, axis=2).astype(np.float32)


if __name__ == "__main__":
    # smoke test with random data
    rng = np.random.default_rng(0)
    x = rng.standard_normal((N, C, H), dtype=np.float32)
    w1 = rng.standard_normal((C, C, H), dtype=np.float32) * 0.02
    w2 = rng.standard_normal((C, C, H), dtype=np.float32) * 0.02
    g1 = np.ones(C, np.float32)
    b1 = np.zeros(C, np.float32)
    g2 = np.ones(C, np.float32)
    b2 = np.zeros(C, np.float32)
    y = kernel(x=x, w1=w1, g1=g1, b1=b1, w2=w2, g2=g2, b2=b2)
    print(y.shape, y.dtype, float(np.abs(y).max()))


# revision 86
# speedup vs baseline: 1.0229x; 1.0094x over previous
"""Trainium2 Bass kernel for nn_BasicBlock1D (locally-connected 1x1 conv x2
with training-mode BatchNorm, residual, ReLU).

Reference computation (per spatial position h, there are H=64 of them):
    out1[n,o,h] = sum_c x[n,c,h] * w1[o,c,h]          (512x512 matmul per h)
    y1 = relu(bn1(out1))                              (stats over (N,H))
    out2[n,o,h] = sum_c y1[n,c,h] * w2[o,c,h]
    y  = relu(bn2(out2) + x)

Sharding: the 64 spatial positions are split across the 8 NeuronCores (8 per
core).  Each core reads only its h-slice of x/w1/w2, so every HBM byte is
read exactly once chip-wide.  BatchNorm statistics span the full (N,H) batch;
each core pre-aggregates its local per-channel (mean, E[x^2]) and a single
4KB AllReduce(add) with a Shared output buffer combines them:
    mean_g = sum_c mean_c / 8,  var_g = sum_c (var_c + mean_c^2)/8 - mean_g^2
(equal-sized groups of 2048 samples per core make this exact).

Schedule (cost-model-driven):
  The kernel is DMA-floor bound: 10MB of inputs + 2MB of output per core at
  the 360 B/ns aggregate DMA bandwidth is ~33us, against ~27us of bf16 PE
  time; the critical path is L1(DMA 6MB) -> bn1 chain -> L2(PE 13.7us) ->
  bn2 chain -> apply/store tail.

  * All inputs stream on the SP HWDGE ring in exact PE-consumption order at
    256KB granularity (x tiles in 2 halves, each 1MB weight tile in 4
    pieces), so layer 1 starts after ~1.1MB is buffered (~4.5us) and runs
    DMA-paced with the PE never stalling (a stalled PE drops out of its
    boosted clock: 394/213/107ns per 256-row matmul cold/mid/boosted, and
    any idle gap resets the boost).
  * g/b/eps constants ride as ONE batched DMA on the otherwise-idle ACT
    ring at t=0 (each DMA costs 625ns of the globally-serialized HWDGE
    sequencer regardless of size, so five separate loads would stall the
    main stream or arrive after the bn1 chain needs them).
  * PE warm-up fillers (garbage matmuls accumulating into a reserved PSUM
    bank) cover the initial prefetch window, and a second filler block is
    emitted contiguously behind layer 1's last matmul so the PE stays in
    its boosted clock state through the bn1-chain gap and layer 2 starts
    back-to-back at full speed.
  * Structure per h-pair: each PSUM tile is a full bank [128, 2, 256]; 8
    matmuls accumulate into it, emitted hh-MAJOR across the chunk's four
    oc tiles so the PE consumes weight pieces in arrival order instead of
    the first oc-group stalling on the chunk's last piece; a single ACT
    op evacuates each tile to bf16
    (walrus rejects GPSIMD touching PSUM at all), and bn_stats reads the
    PSUM bank directly so it runs concurrently with the evacuation.  The
    LAST h-pair's bn_stats are split into two half-bank tuples so the
    first half's stats overlap the second half's matmuls (bn_aggr
    combines tuples of unequal counts exactly).
  * The stats->coefficient chains are emitted PER output-channel chunk in
    the no-collective build (aggr -> (var+eps)^0.5 via a fused pow
    tensor_scalar -> g/std via tensor_tensor divide -> t), all on DVE so
    there are zero cross-engine handoffs; s[oc]/t[oc] unlock downstream
    per-oc work as soon as that oc's raw stats land.  The collective
    build keeps the proven monolithic Sqrt/reciprocal chain around the
    4KB AllReduce.
  * The final phase per tile: DVE affine (s2*o2 + t2, one fused 2-scalar
    tensor_scalar), residual add on DVE TT or Pool TT (walrus accepts
    ONLY the TensorTensor class on Pool -- no tensor_scalar, no max op),
    then relu as ACT Relu or a fused DVE (add 0, max) tensor_scalar for
    the end-tiles where ACT is congested.  Each output (oc, h-half)
    leaves in a 256KB store the moment its two tiles are applied.

A dummy Sqrt at t=0 preloads the ACT function table off the critical
path.

KERNEL_UNROLL=k (default 1) builds the whole pipeline k times back-to-back
into one NEFF -- a measurement aid.

Stack quirks this kernel deliberately avoids (verified empirically on this
axon/PJRT toolchain): tensor_tensor_reduce (faults), tensor_tensor with the
same tile as both operands, DVE memset feeding scalar operands, float
immediates in tensor_scalar, in-place elementwise ops, instructions whose
only output has no reader (walrus drops the alloc and the engine faults),
any non-TensorTensor compute op on Pool/GPSIMD and any Pool access to PSUM
(both fail walrus's per-engine ISA check), and SBUF tiles written by more
than one DMA (the framework under-gates readers of multi-DMA tiles, so
every DMA gets its own tile).
"""

import os
import sys
from contextlib import ExitStack

import numpy as np

_REPO = "/opt/trn_rl_repo"
if _REPO not in sys.path:
    sys.path.insert(0, _REPO)

import ml_dtypes  # noqa: E402

import concourse.bacc as bacc  # noqa: E402
import concourse.tile as tile  # noqa: E402
from concourse import mybir  # noqa: E402
from concourse.bass_utils import run_bass_kernel_spmd  # noqa: E402

N, C, H = 256, 512, 64
NCORES = 8
HS = H // NCORES  # 8 h positions per core
P = 128
KC = C // P  # 4 contraction chunks
OC = C // P  # 4 output-channel chunks
NN = N  # moving free dim of each matmul
HPAIRS = HS // 2  # weight tiles / activation tiles hold 2 h positions
EPS = 1e-5
GB_W = 4 * OC + 8  # packed g/b/cst width

BF16 = mybir.dt.bfloat16
F32 = mybir.dt.float32

LAST_EXEC_NS = None
LAST_RESULTS = None

_cached = None


def _build_program():
    nc = bacc.Bacc(
        "TRN2",
        target_bir_lowering=False,
        debug=False,
        num_devices=NCORES,
    )

    xt_d = nc.dram_tensor("xt", [HPAIRS, P, KC, 2, NN], BF16, kind="ExternalInput")
    w1_d = nc.dram_tensor("w1t", [HPAIRS, P, 2, KC, C], BF16, kind="ExternalInput")
    w2_d = nc.dram_tensor("w2t", [HPAIRS, P, 2, KC, C], BF16, kind="ExternalInput")
    gb_d = nc.dram_tensor("gbt", [P, GB_W], F32, kind="ExternalInput")
    unroll = int(os.environ.get("KERNEL_UNROLL", "1"))
    out_d = nc.dram_tensor(
        "out", [unroll, OC, P, HPAIRS * 2 * NN], BF16, kind="ExternalOutput"
    )
    junk_d = nc.dram_tensor("junk", [P, 2], F32, kind="ExternalOutput")

    add = mybir.AluOpType.add
    mult = mybir.AluOpType.mult
    amax = mybir.AluOpType.max
    AF = mybir.ActivationFunctionType
    use_cc = os.environ.get("KERNEL_NOCC", "0") != "1"

    NF1 = int(os.environ.get("KERNEL_NFILL1", "16"))
    NFM = int(os.environ.get("KERNEL_NFILLM", "20"))

    with tile.TileContext(nc) as tc, ExitStack() as ctx:
        persist = ctx.enter_context(tc.tile_pool(name="persist", bufs=1))
        wpool = ctx.enter_context(tc.tile_pool(name="wpool", bufs=32))
        spool = ctx.enter_context(tc.tile_pool(name="spool", bufs=2))
        psum = ctx.enter_context(tc.tile_pool(name="psum", bufs=7, space="PSUM"))
        psumf = ctx.enter_context(tc.tile_pool(name="psumf", bufs=1, space="PSUM"))
        fpool = ctx.enter_context(tc.tile_pool(name="fpool", bufs=1))
        dram = ctx.enter_context(tc.tile_pool(name="dram", bufs=1, space="DRAM"))

        def hp_tiles(nm, dt):
            return [
                [
                    persist.tile([P, 2, NN], dt, tag=f"{nm}_{k}_{hp}", name=f"{nm}_{k}_{hp}")
                    for hp in range(HPAIRS)
                ]
                for k in range(OC)
            ]

        # --- persistent activations (x in two kc-half tiles per h-pair so
        # the stream gates the first matmuls at 256KB granularity) ---
        xs = [
            [
                persist.tile([P, 2, 2, NN], BF16, tag=f"x{hp}_{kh}", name=f"x{hp}_{kh}")
                for kh in range(2)
            ]
            for hp in range(HPAIRS)
        ]

        def x_at(hp, kc, hh):
            return xs[hp][kc // 2][:, kc % 2, hh, :]
        raw1 = hp_tiles("r1", BF16)   # layer-1 pre-BN output
        y1 = hp_tiles("y1", BF16)
        o2 = hp_tiles("o2", BF16)     # layer-2 pre-BN output

        # g/b/cst: ONE batched DMA on the ACT ring at t=0 (see module doc).
        gbt = persist.tile([P, GB_W], F32, tag="gbt", name="gbt")
        nc.scalar.dma_start(out=gbt, in_=gb_d.ap())
        gbs = {
            "g1": gbt[:, 0 * OC : 1 * OC],
            "b1": gbt[:, 1 * OC : 2 * OC],
            "g2": gbt[:, 2 * OC : 3 * OC],
            "b2": gbt[:, 3 * OC : 4 * OC],
        }
        eps_ap = gbt[:, 4 * OC + 0 : 4 * OC + 1]
        inv8_ap = gbt[:, 4 * OC + 1 : 4 * OC + 2]  # 1/NCORES
        one_ap = gbt[:, 4 * OC + 2 : 4 * OC + 3]   # 1.0
        zero_ap = gbt[:, 4 * OC + 3 : 4 * OC + 4]  # 0.0
        half_ap = gbt[:, 4 * OC + 4 : 4 * OC + 5]  # 0.5

        # ACT function-table preload: a dummy Sqrt as the very first ACT op
        # pulls in the (sqrt + copy/relu/square) table off the critical path.
        # Its junk store is deferred to program end so the ACT SEQ doesn't
        # sit blocked on the HWDGE ring (busy with the input stream).
        dummy_in = persist.tile([P, 1], F32, tag="dmy_i", name="dmy_i")
        nc.vector.memset(dummy_in, 4.0)
        dummy_out = persist.tile([P, 1], F32, tag="dmy_o", name="dmy_o")
        nc.scalar.activation(out=dummy_out, in_=dummy_in, func=AF.Sqrt)

        # --- PE warm-up fillers (see module doc) ---
        filler_w = fpool.tile([P, P], BF16, tag="fil_w", name="fil_w")
        nc.gpsimd.memset(filler_w, 0.0)
        filler_x = fpool.tile([P, NN], BF16, tag="fil_x", name="fil_x")
        nc.gpsimd.memset(filler_x, 0.0)
        fpt = psumf.tile([P, NN], F32, tag="fil_p", name="fil_p")


        def pe_fill(n):
            # self-contained accumulation group per block: a group that
            # spans other groups' matmuls trips the scheduler into
            # serializing it behind unrelated work.  Returns (first, last)
            # instruction handles so callers can pin the block's position
            # in the PE stream with explicit deps (the tile scheduler
            # otherwise hoists dep-free matmuls into any earlier PE idle
            # slot).
            first = last = None
            for i in range(n):
                h = nc.tensor.matmul(
                    fpt, lhsT=filler_w, rhs=filler_x,
                    start=(i == 0), stop=(i == n - 1),
                )
                if first is None:
                    first = h
                last = h
            return first, last

        def w_piece(w_d, hp, hh, kcp):
            # a 256KB piece-TILE of a weight tile, in PE-consumption order:
            # (hh0,kc01) (hh0,kc23) (hh1,kc01) (hh1,kc23).  One DMA per
            # tile: the framework's DMA-completion waits under-gate tiles
            # written by multiple DMAs (verified in the timeline sim), so a
            # piece must be its own tile for readers to be sequenced right.
            wp = wpool.tile([P, 2, C], BF16, tag="wp", name="wp")
            nc.sync.dma_start(
                out=wp, in_=w_d.ap()[hp][:, hh, 2 * kcp : 2 * kcp + 2, :]
            )
            return wp

        def stream_l1(hp):
            # x halves interleaved with w pieces in first-need order
            nc.sync.dma_start(out=xs[hp][0], in_=xt_d.ap()[hp][:, 0:2])
            p00 = w_piece(w1_d, hp, 0, 0)
            p01 = w_piece(w1_d, hp, 0, 1)
            nc.sync.dma_start(out=xs[hp][1], in_=xt_d.ap()[hp][:, 2:4])
            p10 = w_piece(w1_d, hp, 1, 0)
            p11 = w_piece(w1_d, hp, 1, 1)
            return [p00, p01, p10, p11]

        def stream_w2(hp):
            return [w_piece(w2_d, hp, hh, kcp) for hh in range(2) for kcp in range(2)]

        def layer(w_tiles, src_at, dst_tiles, lname, tail_pairs=False):
            """Per-position matmuls + per-(chunk,h-pair) raw BN moments.

            src_at(kc, h) -> [P, NN] AP of the layer input
            dst_tiles[oc][hp][:, hh, :] <- the (h = 2*hp+hh) output slice
            tail_pairs: process the last two h-pairs interleaved by
            oc-PAIRS ((hp2,oc01),(hp3,oc01),(hp2,oc23),(hp3,oc23)) so
            oc0/oc1's raw stats complete ~2.6us before the layer ends and
            their coefficient chains + downstream applies/stores overlap
            the remaining matmuls.
            returns (stats tile [P, OC, HPAIRS, 6], first mm, last mm).
            """
            # HPAIRS+1 raw-stat slots per oc: one PSUM-direct tuple per
            # h-pair for hp<3, and TWO half-bank PSUM tuples for the last
            # h-pair so its first half's stats run while the second half's
            # matmuls are still going (bn_aggr combines tuples of unequal
            # counts exactly).
            st_raw = persist.tile(
                [P, OC, HPAIRS + 1, 6], F32, tag=f"straw_{lname}", name=f"straw_{lname}"
            )
            order = [(hp, oc) for hp in range(HPAIRS) for oc in range(OC)]
            if tail_pairs:
                order = [(hp, oc) for hp in (0, 1) for oc in range(OC)] + [
                    (hp, oc)
                    for ocp in (0, 1)
                    for hp in (2, 3)
                    for oc in (2 * ocp, 2 * ocp + 1)
                ]
            first_mm = last_mm = None
            for hp in range(HPAIRS):
                pieces = w_tiles[hp]
                # hh-MAJOR matmul emission within the chunk: the first 16
                # matmuls need only the (hh0) weight pieces, so the PE
                # consumes the stream as it arrives instead of the first
                # oc-group stalling on the chunk's last piece.  Each
                # pt[:, hh, :] slice is its own accumulation group either
                # way.  For the last h-pair this also completes all hh0
                # half-bank stats inputs 1.7us before the layer ends.
                pts = [None] * OC
                for hh in range(2):
                    h = hp * 2 + hh
                    for oc in range(OC):
                        if pts[oc] is None:
                            # allocate at first use so the pool-rotation
                            # waits stagger instead of bunching at the
                            # chunk boundary
                            pts[oc] = psum.tile([P, 2, NN], F32, tag="ps", name="ps")
                        for kc in range(KC):
                            wp = pieces[2 * hh + kc // 2]
                            m = nc.tensor.matmul(
                                pts[oc][:, hh, :],
                                lhsT=wp[:, kc % 2, oc * P : (oc + 1) * P],
                                rhs=src_at(kc, h),
                                start=(kc == 0),
                                stop=(kc == KC - 1),
                            )
                            if first_mm is None:
                                first_mm = m
                            last_mm = m
                    if hp == HPAIRS - 1:
                        # half-bank stats as soon as this hh's groups close
                        for oc in range(OC):
                            nc.vector.bn_stats(
                                out=st_raw[:, oc, hp + hh, :],
                                in_=pts[oc][:, hh, :],
                            )
                for oc in range(OC):
                    # single-op PSUM evacuation on ACT (the only engine
                    # that can both read PSUM and run concurrently here:
                    # GPSIMD cannot access PSUM at all); bn_stats reads the
                    # PSUM bank directly so it runs concurrently with the
                    # evacuation instead of after it
                    nc.scalar.activation(
                        out=dst_tiles[oc][hp], in_=pts[oc], func=AF.Copy
                    )
                    if hp < HPAIRS - 1:
                        # read the evacuated bf16 tile, not PSUM: the bank
                        # then frees on the ACT evac alone, so the pool
                        # rotation (bufs=7) never stalls a later chunk's
                        # matmuls behind this tile's (lagging) DVE stats
                        nc.vector.bn_stats(
                            out=st_raw[:, oc, hp, :],
                            in_=dst_tiles[oc][hp].rearrange("p a n -> p (a n)"),
                        )
            return st_raw, first_mm, last_mm

        def stats_reduce(st_raw, g_t, b_t, lname, per_oc_done=None):
            """Local aggregate -> 4KB AllReduce(add) -> BN scale/shift.

            bn(v) = s*v + t with s = g/sqrt(var+eps), t = b - mean*s.
            """

            def small(nm, shape=(P, OC)):
                return persist.tile(
                    list(shape), F32, tag=f"{nm}_{lname}", name=f"{nm}_{lname}"
                )

            # Everything below stays on DVE (ACT only for the one Sqrt) --
            # each cross-engine handoff in this serial chain costs ~300ns.
            mv = small("mv", (P, OC, 2))
            if not use_cc:
                # per-oc chains: s[oc]/t[oc] unlock as soon as THAT oc's
                # last raw stats land, so downstream per-oc consumers
                # (y1 h-pair 0, the final apply) start ~1-2us earlier than
                # a monolithic chain would allow.
                std = small("std")
                s_t = small("s")
                mts = small("mts")
                t_t = small("t")
                for oc in range(OC):
                    sl = slice(oc, oc + 1)
                    nc.vector.bn_aggr(out=mv[:, oc, :], in_=st_raw[:, oc, :, :])
                    nc.vector.tensor_scalar(
                        out=std[:, sl], in0=mv[:, oc, 1:2], scalar1=eps_ap,
                        scalar2=half_ap, op0=add, op1=mybir.AluOpType.pow,
                    )
                    nc.vector.tensor_tensor(
                        out=s_t[:, sl], in0=g_t[:, sl], in1=std[:, sl],
                        op=mybir.AluOpType.divide,
                    )
                    nc.vector.tensor_tensor(
                        out=mts[:, sl], in0=mv[:, oc, 0:1], in1=s_t[:, sl],
                        op=mult,
                    )
                    nc.vector.tensor_tensor(
                        out=t_t[:, sl], in0=b_t[:, sl], in1=mts[:, sl],
                        op=mybir.AluOpType.subtract,
                    )
                    if per_oc_done is not None:
                        per_oc_done(oc, s_t, t_t)
                return s_t, t_t
            for oc in range(OC):
                nc.vector.bn_aggr(out=mv[:, oc, :], in_=st_raw[:, oc, :, :])
            if True:
                # pack local (mean, var + mean^2) pairs
                cin = small("cin", (P, OC, 2))
                nc.vector.tensor_scalar(
                    out=cin[:, :, 0], in0=mv[:, :, 0], scalar1=one_ap, scalar2=None,
                    op0=mult,
                )
                msq = small("msq")
                nc.vector.tensor_mul(out=msq, in0=mv[:, :, 0], in1=cin[:, :, 0])
                nc.vector.tensor_add(out=cin[:, :, 1], in0=mv[:, :, 1], in1=msq)
                cci = dram.tile([P, 2 * OC], F32, tag=f"cci_{lname}", name=f"cci_{lname}")
                cco = dram.tile(
                    [P, 2 * OC], F32, tag=f"cco_{lname}", name=f"cco_{lname}",
                    addr_space="Shared",
                )
                nc.scalar.dma_start(out=cci, in_=cin.rearrange("p a b -> p (a b)"))
                nc.gpsimd.collective_compute(
                    "AllReduce",
                    add,
                    replica_groups=[list(range(NCORES))],
                    ins=[cci.opt()],
                    outs=[cco.opt()],
                )
                red = small("red", (P, OC, 2))
                nc.scalar.dma_start(
                    out=red, in_=cco.rearrange("p (a b) -> p a b", a=OC)
                )
                me2 = small("me2", (P, OC, 2))
                nc.vector.tensor_scalar(
                    out=me2, in0=red, scalar1=inv8_ap, scalar2=None, op0=mult
                )
                m_g = me2[:, :, 0]
                mr = small("mr")
                nc.vector.tensor_mul(out=mr, in0=m_g, in1=red[:, :, 0])  # m^2 * 8
                varg = small("varg")  # var * 8
                nc.vector.tensor_sub(out=varg, in0=red[:, :, 1], in1=mr)
                std = small("std")
                nc.scalar.activation(
                    out=std, in_=varg, func=AF.Sqrt, bias=eps_ap, scale=inv8_ap
                )
                rstd = small("rstd")
                nc.vector.reciprocal(out=rstd, in_=std)
                s_t = small("s")
                nc.vector.tensor_mul(out=s_t, in0=rstd, in1=g_t)
            mts = small("mts")
            nc.vector.tensor_mul(out=mts, in0=m_g, in1=s_t)
            t_t = small("t")
            nc.vector.tensor_sub(out=t_t, in0=b_t, in1=mts)
            if per_oc_done is not None:
                for oc in range(OC):
                    per_oc_done(oc, s_t, t_t)
            return s_t, t_t

        from concourse.tile_rust import add_dep_helper

        for _u in range(unroll):
            _, f1_last = pe_fill(NF1)
            # ---- input stream: exact PE-consumption order on the SP ring
            w1_tiles = [stream_l1(hp) for hp in range(HPAIRS)]
            w2_tiles = [stream_w2(hp) for hp in range(HPAIRS)]

            # ---------------- layer 1 ----------------
            stats1, l1_first, l1_last = layer(
                w1_tiles, lambda kc, h: x_at(h // 2, kc, h % 2), raw1, f"l1_{_u}",
            )
            # keep the PE boosted through the bn1-chain gap: fillers run
            # back-to-back behind layer 1's last matmul, and layer 2's
            # first matmul queues back-to-back behind the fillers.  The
            # tile scheduler hoists dep-free matmuls into any earlier PE
            # idle slot, so the block is pinned with explicit deps.
            if f1_last is not None:
                add_dep_helper(l1_first.ins, f1_last.ins, reason="NF1 before L1")
            fm_first, fm_last = pe_fill(NFM)
            if fm_first is not None:
                add_dep_helper(fm_first.ins, l1_last.ins, reason="mid fillers after L1")
            # y1 = relu(s1*out1 + t1).  The h-pair-0 tile for each oc is
            # emitted from the per-oc chain hook so the first layer-2
            # matmul group unblocks as early as possible.  Engine split
            # balances hp0 latency (ACT 1-op + DVE 2-op) and mid-L2
            # occupancy (ACT/DVE/Pool); Pool tiles go on late h-pairs
            # since its 2-op path is ~3.2us for 2 tiles.
            Y1_ACT = {(2, 0), (3, 0), (0, 1), (1, 1), (0, 2), (1, 2), (0, 3)}

            def y1_tile(oc, hp, s1, t1):
                if (oc, hp) in Y1_ACT:
                    nc.scalar.activation(
                        out=y1[oc][hp],
                        in_=raw1[oc][hp],
                        func=AF.Relu,
                        scale=s1[:, oc : oc + 1],
                        bias=t1[:, oc : oc + 1],
                    )
                else:
                    ytmp = spool.tile([P, 2, NN], BF16, tag="ya", name="ya", bufs=3)
                    nc.vector.tensor_scalar(
                        out=ytmp,
                        in0=raw1[oc][hp],
                        scalar1=s1[:, oc : oc + 1],
                        scalar2=t1[:, oc : oc + 1],
                        op0=mult,
                        op1=add,
                    )
                    nc.vector.tensor_scalar(
                        out=y1[oc][hp], in0=ytmp, scalar1=zero_ap,
                        scalar2=None, op0=amax,
                    )

            hook1 = (
                (lambda oc, s, t: y1_tile(oc, 0, s, t))
                if os.environ.get("KERNEL_Y1_HOOK", "0") == "1"
                else None
            )
            s1, t1 = stats_reduce(
                stats1, gbs["g1"], gbs["b1"], f"l1_{_u}", per_oc_done=hook1
            )
            for hp in range(0 if hook1 is None else 1, HPAIRS):
                for oc in range(OC):
                    y1_tile(oc, hp, s1, t1)

            # ---------------- layer 2 ----------------
            stats2, l2_first, l2_last = layer(
                w2_tiles, lambda kc, h: y1[kc][h // 2][:, h % 2, :], o2, f"l2_{_u}",
            )
            if fm_last is not None:
                add_dep_helper(l2_first.ins, fm_last.ins, reason="L2 after mid fillers")
            # y = relu(s2*out2 + t2 + x): DVE affine TSP (194), then the
            # residual add on DVE TT (327) or Pool TT (1111, PATH_C), then
            # relu on ACT (612) or DVE fused (add,max) TSP (194, PATH_A).
            # Stores leave per (oc, half) as soon as the half is applied.
            PATH_C = {(0, 1), (1, 1), (2, 1), (3, 1), (0, 2), (1, 2)}
            PATH_A = {(2, 2), (3, 2), (0, 3), (1, 3), (2, 3), (3, 3)}
            outbigs = [
                persist.tile(
                    [P, HPAIRS, 2, NN], BF16, tag=f"obig_{oc}", name=f"obig_{oc}"
                )
                for oc in range(OC)
            ]

            def apply_tile(oc, hp, s2, t2):
                outbig = outbigs[oc]
                s2c = s2[:, oc : oc + 1]
                t2c = t2[:, oc : oc + 1]
                x_in = xs[hp][oc // 2][:, oc % 2, :, :]
                aff = spool.tile([P, 2, NN], BF16, tag="f1", name="f1", bufs=8)
                nc.vector.tensor_scalar(
                    out=aff, in0=o2[oc][hp], scalar1=s2c, scalar2=t2c,
                    op0=mult, op1=add,
                )
                v = spool.tile([P, 2, NN], BF16, tag="f2", name="f2", bufs=8)
                e_add = nc.gpsimd if (oc, hp) in PATH_C else nc.vector
                e_add.tensor_tensor(out=v, in0=aff, in1=x_in, op=add)
                if (oc, hp) in PATH_A:
                    nc.vector.tensor_scalar(
                        out=outbig[:, hp, :, :], in0=v, scalar1=zero_ap,
                        scalar2=None, op0=amax,
                    )
                else:
                    nc.scalar.activation(
                        out=outbig[:, hp, :, :], in_=v, func=AF.Relu
                    )

            half = 2 * NN

            def apply_oc(oc, s2, t2):
                for hf in range(2):
                    apply_tile(oc, 2 * hf, s2, t2)
                    apply_tile(oc, 2 * hf + 1, s2, t2)
                    nc.sync.dma_start(
                        out=out_d.ap()[_u, oc][:, 2 * hf * half : (2 * hf + 2) * half],
                        in_=outbigs[oc][:, 2 * hf : 2 * hf + 2, :, :].rearrange(
                            "p a b n -> p (a b n)"
                        ),
                    )

            if os.environ.get("KERNEL_APPLY_HOOK", "1") == "1":
                stats_reduce(
                    stats2, gbs["g2"], gbs["b2"], f"l2_{_u}", per_oc_done=apply_oc
                )
            else:
                s2, t2 = stats_reduce(stats2, gbs["g2"], gbs["b2"], f"l2_{_u}")
                for hf in range(2):
                    for oc in range(OC):
                        apply_tile(oc, 2 * hf, s2, t2)
                        apply_tile(oc, 2 * hf + 1, s2, t2)
                        nc.sync.dma_start(
                            out=out_d.ap()[_u, oc][
                                :, 2 * hf * half : (2 * hf + 2) * half
                            ],
                            in_=outbigs[oc][:, 2 * hf : 2 * hf + 2, :, :].rearrange(
                                "p a b n -> p (a b n)"
                            ),
                        )

        nc.scalar.dma_start(out=junk_d.ap()[:, 0:1], in_=dummy_out)
        if NF1 + NFM > 0:
            jnk2 = persist.tile([P, 1], F32, tag="jnk2", name="jnk2")
            nc.scalar.activation(out=jnk2, in_=fpt[:, 0:1], func=AF.Copy)
            nc.scalar.dma_start(out=junk_d.ap()[:, 1:2], in_=jnk2)

    nc.compile()
    return nc


def _get_program():
    global _cached
    if _cached is None:
        _cached = _build_program()
    return _cached


def _pack_inputs(x, w1, g1, b1, w2, g2, b2):
    """Host-side shard + repack into the device layouts (see module doc)."""
    bf16 = ml_dtypes.bfloat16
    # x: (N, C, H) -> (KC, P, H, N) -> per-core (HPAIRS, P, KC, 2, N)
    xt = np.ascontiguousarray(x.transpose(1, 2, 0)).reshape(KC, P, H, N)
    xt = xt.astype(bf16)

    # w: (O, C, H) -> (H, P, KC, C)
    def packw(w):
        wt = w.transpose(2, 1, 0).reshape(H, KC, P, C).transpose(0, 2, 1, 3)
        return wt.astype(bf16)

    w1t = packw(w1)
    w2t = packw(w2)

    def packg(v):
        return np.ascontiguousarray(v.reshape(OC, P).T.astype(np.float32))

    gbt = np.empty((P, GB_W), np.float32)
    gbt[:, 0 * OC : 1 * OC] = packg(g1)
    gbt[:, 1 * OC : 2 * OC] = packg(b1)
    gbt[:, 2 * OC : 3 * OC] = packg(g2)
    gbt[:, 3 * OC : 4 * OC] = packg(b2)
    gbt[:, 4 * OC + 0] = EPS
    gbt[:, 4 * OC + 1] = 1.0 / NCORES
    gbt[:, 4 * OC + 2] = 1.0
    gbt[:, 4 * OC + 3] = 0.0
    gbt[:, 4 * OC + 4] = 0.5
    gbt[:, 4 * OC + 5 :] = 0.0

    in_maps = []
    for c in range(NCORES):
        h0, h1 = c * HS, (c + 1) * HS
        # (KC, P, HS, N) -> (KC, P, HPAIRS, 2, N) -> (HPAIRS, P, KC, 2, N)
        xc = xt[:, :, h0:h1, :].reshape(KC, P, HPAIRS, 2, N)
        xc = np.ascontiguousarray(xc.transpose(2, 1, 0, 3, 4))
        in_maps.append(
            {
                "xt": xc,
                "w1t": np.ascontiguousarray(w1t[h0:h1]).reshape(
                    HPAIRS, 2, P, KC, C
                ).transpose(0, 2, 1, 3, 4).copy(),
                "w2t": np.ascontiguousarray(w2t[h0:h1]).reshape(
                    HPAIRS, 2, P, KC, C
                ).transpose(0, 2, 1, 3, 4).copy(),
                "gbt": gbt,
            }
        )
    return in_maps


def kernel(x, w1, g1, b1, w2, g2, b2):
    global LAST_EXEC_NS, LAST_RESULTS
    nc = _get_program()
    in_maps = _pack_inputs(
        np.asarray(x, dtype=np.float32),
        np.asarray(w1, dtype=np.float32),
        np.asarray(g1, dtype=np.float32),
        np.asarray(b1, dtype=np.float32),
        np.asarray(w2, dtype=np.float32),
        np.asarray(g2, dtype=np.float32),
        np.asarray(b2, dtype=np.float32),
    )
    trace = os.environ.get("KERNEL_TRACE", "0") == "1"
    res = run_bass_kernel_spmd(
        nc, in_maps, list(range(NCORES)), trace=trace
    )
    LAST_EXEC_NS = res.exec_time_ns
    LAST_RESULTS = res
    parts = []
    for c in range(NCORES):
        r = np.asarray(res.results[c]["out"])[0]  # [oc, p, (hp hh n)] bf16
        r = r.astype(np.float32).reshape(OC, P, HPAIRS, 2, N)
        # -> (n, c, h): c = oc*P + p, h = 2*hp + hh
        r = r.transpose(4, 0, 1, 2, 3).reshape(N, C, HS)
        parts.append(r)
    return np.concatenate(parts, axis=2).astype(np.float32)


if __name__ == "__main__":
    # smoke test with random data
    rng = np.random.default_rng(0)
    x = rng.standard_normal((N, C, H), dtype=np.float32)
    w1 = rng.standard_normal((C, C, H), dtype=np.float32) * 0.02
    w2 = rng.standard_normal((C, C, H), dtype=np.float32) * 0.02
    g1 = np.ones(C, np.float32)
    b1 = np.zeros(C, np.float32)
    g2 = np.ones(C, np.float32)
    b2 = np.zeros(C, np.float32)
    y = kernel(x=x, w1=w1, g1=g1, b1=b1, w2=w2, g2=g2, b2=b2)
    print(y.shape, y.dtype, float(np.abs(y).max()))


# revision 90
# speedup vs baseline: 1.0243x; 1.0014x over previous
"""Trainium2 Bass kernel for nn_BasicBlock1D (locally-connected 1x1 conv x2
with training-mode BatchNorm, residual, ReLU).

Reference computation (per spatial position h, there are H=64 of them):
    out1[n,o,h] = sum_c x[n,c,h] * w1[o,c,h]          (512x512 matmul per h)
    y1 = relu(bn1(out1))                              (stats over (N,H))
    out2[n,o,h] = sum_c y1[n,c,h] * w2[o,c,h]
    y  = relu(bn2(out2) + x)

Sharding: the 64 spatial positions are split across the 8 NeuronCores (8 per
core).  Each core reads only its h-slice of x/w1/w2, so every HBM byte is
read exactly once chip-wide.  BatchNorm statistics span the full (N,H) batch;
each core pre-aggregates its local per-channel (mean, E[x^2]) and a single
4KB AllReduce(add) with a Shared output buffer combines them:
    mean_g = sum_c mean_c / 8,  var_g = sum_c (var_c + mean_c^2)/8 - mean_g^2
(equal-sized groups of 2048 samples per core make this exact).

Schedule (cost-model-driven):
  The kernel is DMA-floor bound: 10MB of inputs + 2MB of output per core at
  the 360 B/ns aggregate DMA bandwidth is ~33us, against ~27us of bf16 PE
  time; the critical path is L1(DMA 6MB) -> bn1 chain -> L2(PE 13.7us) ->
  bn2 chain -> apply/store tail.

  * All inputs stream on the SP HWDGE ring in exact PE-consumption order at
    256KB granularity (x tiles in 2 halves, each 1MB weight tile in 4
    pieces), so layer 1 starts after ~1.1MB is buffered (~4.5us) and runs
    DMA-paced with the PE never stalling (a stalled PE drops out of its
    boosted clock: 394/213/107ns per 256-row matmul cold/mid/boosted, and
    any idle gap resets the boost).
  * g/b/eps constants ride as ONE batched DMA on the otherwise-idle ACT
    ring at t=0 (each DMA costs 625ns of the globally-serialized HWDGE
    sequencer regardless of size, so five separate loads would stall the
    main stream or arrive after the bn1 chain needs them).
  * PE warm-up fillers (garbage matmuls accumulating into a reserved PSUM
    bank) cover the initial prefetch window, and a second filler block is
    emitted contiguously behind layer 1's last matmul so the PE stays in
    its boosted clock state through the bn1-chain gap and layer 2 starts
    back-to-back at full speed.
  * Structure per h-pair: each PSUM tile is a full bank [128, 2, 256]; 8
    matmuls accumulate into it, emitted hh-MAJOR across the chunk's four
    oc tiles so the PE consumes weight pieces in arrival order instead of
    the first oc-group stalling on the chunk's last piece; a single ACT
    op evacuates each tile to bf16 (walrus rejects GPSIMD touching PSUM
    at all).  bn_stats reads the EVACUATED bf16 tile so the PSUM bank
    frees on the evac alone and the pool rotation (bufs=7) never stalls
    a later chunk's matmuls behind lagging DVE stats -- except the LAST
    h-pair, whose stats read PSUM directly (no evac latency on the
    chain-critical path) as two half-bank tuples so the first half's
    stats overlap the second half's matmuls (bn_aggr combines tuples of
    unequal counts exactly).
  * The stats->coefficient chains are emitted PER output-channel chunk in
    the no-collective build (aggr -> (var+eps)^0.5 via a fused pow
    tensor_scalar -> g/std via tensor_tensor divide -> t), all on DVE so
    there are zero cross-engine handoffs; s[oc]/t[oc] unlock downstream
    per-oc work as soon as that oc's raw stats land.  The collective
    build keeps the proven monolithic Sqrt/reciprocal chain around the
    4KB AllReduce.
  * The final phase per tile: DVE affine (s2*o2 + t2, one fused 2-scalar
    tensor_scalar), residual add on DVE TT or Pool TT (walrus accepts
    ONLY the TensorTensor class on Pool -- no tensor_scalar, no max op),
    then relu as ACT Relu or a fused DVE (add 0, max) tensor_scalar for
    the end-tiles where ACT is congested.  Each output (oc, h-half)
    leaves in a 256KB store the moment its two tiles are applied.

A dummy Sqrt at t=0 preloads the ACT function table off the critical
path.

KERNEL_UNROLL=k (default 1) builds the whole pipeline k times back-to-back
into one NEFF -- a measurement aid.

Stack quirks this kernel deliberately avoids (verified empirically on this
axon/PJRT toolchain): tensor_tensor_reduce (faults), tensor_tensor with the
same tile as both operands, DVE memset feeding scalar operands, float
immediates in tensor_scalar, in-place elementwise ops, instructions whose
only output has no reader (walrus drops the alloc and the engine faults),
any non-TensorTensor compute op on Pool/GPSIMD and any Pool access to PSUM
(both fail walrus's per-engine ISA check), and SBUF tiles written by more
than one DMA (the framework under-gates readers of multi-DMA tiles, so
every DMA gets its own tile).
"""

import os
import sys
from contextlib import ExitStack

import numpy as np

_REPO = "/opt/trn_rl_repo"
if _REPO not in sys.path:
    sys.path.insert(0, _REPO)

import ml_dtypes  # noqa: E402

import concourse.bacc as bacc  # noqa: E402
import concourse.tile as tile  # noqa: E402
from concourse import mybir  # noqa: E402
from concourse.bass_utils import run_bass_kernel_spmd  # noqa: E402

N, C, H = 256, 512, 64
NCORES = 8
HS = H // NCORES  # 8 h positions per core
P = 128
KC = C // P  # 4 contraction chunks
OC = C // P  # 4 output-channel chunks
NN = N  # moving free dim of each matmul
HPAIRS = HS // 2  # weight tiles / activation tiles hold 2 h positions
EPS = 1e-5
GB_W = 4 * OC + 8  # packed g/b/cst width

BF16 = mybir.dt.bfloat16
F32 = mybir.dt.float32

LAST_EXEC_NS = None
LAST_RESULTS = None

_cached = None


def _build_program():
    nc = bacc.Bacc(
        "TRN2",
        target_bir_lowering=False,
        debug=False,
        num_devices=NCORES,
    )

    xt_d = nc.dram_tensor("xt", [HPAIRS, P, KC, 2, NN], BF16, kind="ExternalInput")
    w1_d = nc.dram_tensor("w1t", [HPAIRS, P, 2, KC, C], BF16, kind="ExternalInput")
    w2_d = nc.dram_tensor("w2t", [HPAIRS, P, 2, KC, C], BF16, kind="ExternalInput")
    gb_d = nc.dram_tensor("gbt", [P, GB_W], F32, kind="ExternalInput")
    unroll = int(os.environ.get("KERNEL_UNROLL", "1"))
    out_d = nc.dram_tensor(
        "out", [unroll, OC, P, HPAIRS * 2 * NN], BF16, kind="ExternalOutput"
    )
    junk_d = nc.dram_tensor("junk", [P, 2], F32, kind="ExternalOutput")

    add = mybir.AluOpType.add
    mult = mybir.AluOpType.mult
    amax = mybir.AluOpType.max
    AF = mybir.ActivationFunctionType
    use_cc = os.environ.get("KERNEL_NOCC", "0") != "1"

    NF1 = int(os.environ.get("KERNEL_NFILL1", "16"))
    NFM = int(os.environ.get("KERNEL_NFILLM", "16"))

    with tile.TileContext(nc) as tc, ExitStack() as ctx:
        persist = ctx.enter_context(tc.tile_pool(name="persist", bufs=1))
        wpool = ctx.enter_context(tc.tile_pool(name="wpool", bufs=32))
        spool = ctx.enter_context(tc.tile_pool(name="spool", bufs=2))
        psum = ctx.enter_context(tc.tile_pool(name="psum", bufs=7, space="PSUM"))
        psumf = ctx.enter_context(tc.tile_pool(name="psumf", bufs=1, space="PSUM"))
        fpool = ctx.enter_context(tc.tile_pool(name="fpool", bufs=1))
        dram = ctx.enter_context(tc.tile_pool(name="dram", bufs=1, space="DRAM"))

        def hp_tiles(nm, dt):
            return [
                [
                    persist.tile([P, 2, NN], dt, tag=f"{nm}_{k}_{hp}", name=f"{nm}_{k}_{hp}")
                    for hp in range(HPAIRS)
                ]
                for k in range(OC)
            ]

        # --- persistent activations (x in two kc-half tiles per h-pair so
        # the stream gates the first matmuls at 256KB granularity) ---
        xs = [
            [
                persist.tile([P, 2, 2, NN], BF16, tag=f"x{hp}_{kh}", name=f"x{hp}_{kh}")
                for kh in range(2)
            ]
            for hp in range(HPAIRS)
        ]

        def x_at(hp, kc, hh):
            return xs[hp][kc // 2][:, kc % 2, hh, :]
        raw1 = hp_tiles("r1", BF16)   # layer-1 pre-BN output
        y1 = hp_tiles("y1", BF16)
        o2 = hp_tiles("o2", BF16)     # layer-2 pre-BN output

        # g/b/cst: ONE batched DMA on the ACT ring at t=0 (see module doc).
        gbt = persist.tile([P, GB_W], F32, tag="gbt", name="gbt")
        nc.scalar.dma_start(out=gbt, in_=gb_d.ap())
        gbs = {
            "g1": gbt[:, 0 * OC : 1 * OC],
            "b1": gbt[:, 1 * OC : 2 * OC],
            "g2": gbt[:, 2 * OC : 3 * OC],
            "b2": gbt[:, 3 * OC : 4 * OC],
        }
        eps_ap = gbt[:, 4 * OC + 0 : 4 * OC + 1]
        inv8_ap = gbt[:, 4 * OC + 1 : 4 * OC + 2]  # 1/NCORES
        one_ap = gbt[:, 4 * OC + 2 : 4 * OC + 3]   # 1.0
        zero_ap = gbt[:, 4 * OC + 3 : 4 * OC + 4]  # 0.0
        half_ap = gbt[:, 4 * OC + 4 : 4 * OC + 5]  # 0.5

        # ACT function-table preload: a dummy Sqrt as the very first ACT op
        # pulls in the (sqrt + copy/relu/square) table off the critical path.
        # Its junk store is deferred to program end so the ACT SEQ doesn't
        # sit blocked on the HWDGE ring (busy with the input stream).
        dummy_in = persist.tile([P, 1], F32, tag="dmy_i", name="dmy_i")
        nc.vector.memset(dummy_in, 4.0)
        dummy_out = persist.tile([P, 1], F32, tag="dmy_o", name="dmy_o")
        nc.scalar.activation(out=dummy_out, in_=dummy_in, func=AF.Sqrt)

        # --- PE warm-up fillers (see module doc) ---
        filler_w = fpool.tile([P, P], BF16, tag="fil_w", name="fil_w")
        nc.gpsimd.memset(filler_w, 0.0)
        filler_x = fpool.tile([P, NN], BF16, tag="fil_x", name="fil_x")
        nc.gpsimd.memset(filler_x, 0.0)
        fpt = psumf.tile([P, NN], F32, tag="fil_p", name="fil_p")


        def pe_fill(n):
            # self-contained accumulation group per block: a group that
            # spans other groups' matmuls trips the scheduler into
            # serializing it behind unrelated work.  Returns (first, last)
            # instruction handles so callers can pin the block's position
            # in the PE stream with explicit deps (the tile scheduler
            # otherwise hoists dep-free matmuls into any earlier PE idle
            # slot).
            first = last = None
            for i in range(n):
                h = nc.tensor.matmul(
                    fpt, lhsT=filler_w, rhs=filler_x,
                    start=(i == 0), stop=(i == n - 1),
                )
                if first is None:
                    first = h
                last = h
            return first, last

        def w_piece(w_d, hp, hh, kcp):
            # a 256KB piece-TILE of a weight tile, in PE-consumption order:
            # (hh0,kc01) (hh0,kc23) (hh1,kc01) (hh1,kc23).  One DMA per
            # tile: the framework's DMA-completion waits under-gate tiles
            # written by multiple DMAs (verified in the timeline sim), so a
            # piece must be its own tile for readers to be sequenced right.
            wp = wpool.tile([P, 2, C], BF16, tag="wp", name="wp")
            nc.sync.dma_start(
                out=wp, in_=w_d.ap()[hp][:, hh, 2 * kcp : 2 * kcp + 2, :]
            )
            return wp

        def stream_l1(hp):
            # x halves interleaved with w pieces in first-need order
            nc.sync.dma_start(out=xs[hp][0], in_=xt_d.ap()[hp][:, 0:2])
            p00 = w_piece(w1_d, hp, 0, 0)
            p01 = w_piece(w1_d, hp, 0, 1)
            nc.sync.dma_start(out=xs[hp][1], in_=xt_d.ap()[hp][:, 2:4])
            p10 = w_piece(w1_d, hp, 1, 0)
            p11 = w_piece(w1_d, hp, 1, 1)
            return [p00, p01, p10, p11]

        def stream_w2(hp):
            return [w_piece(w2_d, hp, hh, kcp) for hh in range(2) for kcp in range(2)]

        def layer(w_tiles, src_at, dst_tiles, lname, tail_pairs=False):
            """Per-position matmuls + per-(chunk,h-pair) raw BN moments.

            src_at(kc, h) -> [P, NN] AP of the layer input
            dst_tiles[oc][hp][:, hh, :] <- the (h = 2*hp+hh) output slice
            tail_pairs: process the last two h-pairs interleaved by
            oc-PAIRS ((hp2,oc01),(hp3,oc01),(hp2,oc23),(hp3,oc23)) so
            oc0/oc1's raw stats complete ~2.6us before the layer ends and
            their coefficient chains + downstream applies/stores overlap
            the remaining matmuls.
            returns (stats tile [P, OC, HPAIRS, 6], first mm, last mm).
            """
            # HPAIRS+1 raw-stat slots per oc: one PSUM-direct tuple per
            # h-pair for hp<3, and TWO half-bank PSUM tuples for the last
            # h-pair so its first half's stats run while the second half's
            # matmuls are still going (bn_aggr combines tuples of unequal
            # counts exactly).
            st_raw = persist.tile(
                [P, OC, HPAIRS + 1, 6], F32, tag=f"straw_{lname}", name=f"straw_{lname}"
            )
            order = [(hp, oc) for hp in range(HPAIRS) for oc in range(OC)]
            if tail_pairs:
                order = [(hp, oc) for hp in (0, 1) for oc in range(OC)] + [
                    (hp, oc)
                    for ocp in (0, 1)
                    for hp in (2, 3)
                    for oc in (2 * ocp, 2 * ocp + 1)
                ]
            first_mm = last_mm = None
            for hp in range(HPAIRS):
                pieces = w_tiles[hp]
                # hh-MAJOR matmul emission within the chunk: the first 16
                # matmuls need only the (hh0) weight pieces, so the PE
                # consumes the stream as it arrives instead of the first
                # oc-group stalling on the chunk's last piece.  Each
                # pt[:, hh, :] slice is its own accumulation group either
                # way.  For the last h-pair this also completes all hh0
                # half-bank stats inputs 1.7us before the layer ends.
                pts = [None] * OC
                for hh in range(2):
                    h = hp * 2 + hh
                    for oc in range(OC):
                        if pts[oc] is None:
                            # allocate at first use so the pool-rotation
                            # waits stagger instead of bunching at the
                            # chunk boundary
                            pts[oc] = psum.tile([P, 2, NN], F32, tag="ps", name="ps")
                        for kc in range(KC):
                            wp = pieces[2 * hh + kc // 2]
                            m = nc.tensor.matmul(
                                pts[oc][:, hh, :],
                                lhsT=wp[:, kc % 2, oc * P : (oc + 1) * P],
                                rhs=src_at(kc, h),
                                start=(kc == 0),
                                stop=(kc == KC - 1),
                            )
                            if first_mm is None:
                                first_mm = m
                            last_mm = m
                    if hp == HPAIRS - 1:
                        # half-bank stats as soon as this hh's groups close
                        for oc in range(OC):
                            nc.vector.bn_stats(
                                out=st_raw[:, oc, hp + hh, :],
                                in_=pts[oc][:, hh, :],
                            )
                for oc in range(OC):
                    # single-op PSUM evacuation on ACT (the only engine
                    # that can both read PSUM and run concurrently here:
                    # GPSIMD cannot access PSUM at all); bn_stats reads the
                    # PSUM bank directly so it runs concurrently with the
                    # evacuation instead of after it
                    nc.scalar.activation(
                        out=dst_tiles[oc][hp], in_=pts[oc], func=AF.Copy
                    )
                    if hp < HPAIRS - 1:
                        # read the evacuated bf16 tile, not PSUM: the bank
                        # then frees on the ACT evac alone, so the pool
                        # rotation (bufs=7) never stalls a later chunk's
                        # matmuls behind this tile's (lagging) DVE stats
                        nc.vector.bn_stats(
                            out=st_raw[:, oc, hp, :],
                            in_=dst_tiles[oc][hp].rearrange("p a n -> p (a n)"),
                        )
            return st_raw, first_mm, last_mm

        def stats_reduce(st_raw, g_t, b_t, lname, per_oc_done=None):
            """Local aggregate -> 4KB AllReduce(add) -> BN scale/shift.

            bn(v) = s*v + t with s = g/sqrt(var+eps), t = b - mean*s.
            """

            def small(nm, shape=(P, OC)):
                return persist.tile(
                    list(shape), F32, tag=f"{nm}_{lname}", name=f"{nm}_{lname}"
                )

            # Everything below stays on DVE (ACT only for the one Sqrt) --
            # each cross-engine handoff in this serial chain costs ~300ns.
            mv = small("mv", (P, OC, 2))
            if not use_cc:
                # per-oc chains: s[oc]/t[oc] unlock as soon as THAT oc's
                # last raw stats land, so downstream per-oc consumers
                # (y1 h-pair 0, the final apply) start ~1-2us earlier than
                # a monolithic chain would allow.
                std = small("std")
                s_t = small("s")
                mts = small("mts")
                t_t = small("t")
                for oc in range(OC):
                    sl = slice(oc, oc + 1)
                    nc.vector.bn_aggr(out=mv[:, oc, :], in_=st_raw[:, oc, :, :])
                    nc.vector.tensor_scalar(
                        out=std[:, sl], in0=mv[:, oc, 1:2], scalar1=eps_ap,
                        scalar2=half_ap, op0=add, op1=mybir.AluOpType.pow,
                    )
                    nc.vector.tensor_tensor(
                        out=s_t[:, sl], in0=g_t[:, sl], in1=std[:, sl],
                        op=mybir.AluOpType.divide,
                    )
                    nc.vector.tensor_tensor(
                        out=mts[:, sl], in0=mv[:, oc, 0:1], in1=s_t[:, sl],
                        op=mult,
                    )
                    nc.vector.tensor_tensor(
                        out=t_t[:, sl], in0=b_t[:, sl], in1=mts[:, sl],
                        op=mybir.AluOpType.subtract,
                    )
                    if per_oc_done is not None:
                        per_oc_done(oc, s_t, t_t)
                return s_t, t_t
            for oc in range(OC):
                nc.vector.bn_aggr(out=mv[:, oc, :], in_=st_raw[:, oc, :, :])
            if True:
                # pack local (mean, var + mean^2) pairs
                cin = small("cin", (P, OC, 2))
                nc.vector.tensor_scalar(
                    out=cin[:, :, 0], in0=mv[:, :, 0], scalar1=one_ap, scalar2=None,
                    op0=mult,
                )
                msq = small("msq")
                nc.vector.tensor_mul(out=msq, in0=mv[:, :, 0], in1=cin[:, :, 0])
                nc.vector.tensor_add(out=cin[:, :, 1], in0=mv[:, :, 1], in1=msq)
                cci = dram.tile([P, 2 * OC], F32, tag=f"cci_{lname}", name=f"cci_{lname}")
                cco = dram.tile(
                    [P, 2 * OC], F32, tag=f"cco_{lname}", name=f"cco_{lname}",
                    addr_space="Shared",
                )
                nc.scalar.dma_start(out=cci, in_=cin.rearrange("p a b -> p (a b)"))
                nc.gpsimd.collective_compute(
                    "AllReduce",
                    add,
                    replica_groups=[list(range(NCORES))],
                    ins=[cci.opt()],
                    outs=[cco.opt()],
                )
                red = small("red", (P, OC, 2))
                nc.scalar.dma_start(
                    out=red, in_=cco.rearrange("p (a b) -> p a b", a=OC)
                )
                me2 = small("me2", (P, OC, 2))
                nc.vector.tensor_scalar(
                    out=me2, in0=red, scalar1=inv8_ap, scalar2=None, op0=mult
                )
                m_g = me2[:, :, 0]
                mr = small("mr")
                nc.vector.tensor_mul(out=mr, in0=m_g, in1=red[:, :, 0])  # m^2 * 8
                varg = small("varg")  # var * 8
                nc.vector.tensor_sub(out=varg, in0=red[:, :, 1], in1=mr)
                std = small("std")
                nc.scalar.activation(
                    out=std, in_=varg, func=AF.Sqrt, bias=eps_ap, scale=inv8_ap
                )
                rstd = small("rstd")
                nc.vector.reciprocal(out=rstd, in_=std)
                s_t = small("s")
                nc.vector.tensor_mul(out=s_t, in0=rstd, in1=g_t)
            mts = small("mts")
            nc.vector.tensor_mul(out=mts, in0=m_g, in1=s_t)
            t_t = small("t")
            nc.vector.tensor_sub(out=t_t, in0=b_t, in1=mts)
            if per_oc_done is not None:
                for oc in range(OC):
                    per_oc_done(oc, s_t, t_t)
            return s_t, t_t

        from concourse.tile_rust import add_dep_helper

        for _u in range(unroll):
            _, f1_last = pe_fill(NF1)
            # ---- input stream: exact PE-consumption order on the SP ring
            w1_tiles = [stream_l1(hp) for hp in range(HPAIRS)]
            w2_tiles = [stream_w2(hp) for hp in range(HPAIRS)]

            # ---------------- layer 1 ----------------
            stats1, l1_first, l1_last = layer(
                w1_tiles, lambda kc, h: x_at(h // 2, kc, h % 2), raw1, f"l1_{_u}",
            )
            # keep the PE boosted through the bn1-chain gap: fillers run
            # back-to-back behind layer 1's last matmul, and layer 2's
            # first matmul queues back-to-back behind the fillers.  The
            # tile scheduler hoists dep-free matmuls into any earlier PE
            # idle slot, so the block is pinned with explicit deps.
            if f1_last is not None:
                add_dep_helper(l1_first.ins, f1_last.ins, reason="NF1 before L1")
            fm_first, fm_last = pe_fill(NFM)
            if fm_first is not None:
                add_dep_helper(fm_first.ins, l1_last.ins, reason="mid fillers after L1")
            # y1 = relu(s1*out1 + t1).  The h-pair-0 tile for each oc is
            # emitted from the per-oc chain hook so the first layer-2
            # matmul group unblocks as early as possible.  Engine split
            # balances hp0 latency (ACT 1-op + DVE 2-op) and mid-L2
            # occupancy (ACT/DVE/Pool); Pool tiles go on late h-pairs
            # since its 2-op path is ~3.2us for 2 tiles.
            Y1_ACT = {(2, 0), (3, 0), (0, 1), (1, 1), (0, 2), (1, 2), (0, 3)}

            def y1_tile(oc, hp, s1, t1):
                if (oc, hp) in Y1_ACT:
                    nc.scalar.activation(
                        out=y1[oc][hp],
                        in_=raw1[oc][hp],
                        func=AF.Relu,
                        scale=s1[:, oc : oc + 1],
                        bias=t1[:, oc : oc + 1],
                    )
                else:
                    ytmp = spool.tile([P, 2, NN], BF16, tag="ya", name="ya", bufs=3)
                    nc.vector.tensor_scalar(
                        out=ytmp,
                        in0=raw1[oc][hp],
                        scalar1=s1[:, oc : oc + 1],
                        scalar2=t1[:, oc : oc + 1],
                        op0=mult,
                        op1=add,
                    )
                    nc.vector.tensor_scalar(
                        out=y1[oc][hp], in0=ytmp, scalar1=zero_ap,
                        scalar2=None, op0=amax,
                    )

            hook1 = (
                (lambda oc, s, t: y1_tile(oc, 0, s, t))
                if os.environ.get("KERNEL_Y1_HOOK", "0") == "1"
                else None
            )
            s1, t1 = stats_reduce(
                stats1, gbs["g1"], gbs["b1"], f"l1_{_u}", per_oc_done=hook1
            )
            for hp in range(0 if hook1 is None else 1, HPAIRS):
                for oc in range(OC):
                    y1_tile(oc, hp, s1, t1)

            # ---------------- layer 2 ----------------
            stats2, l2_first, l2_last = layer(
                w2_tiles, lambda kc, h: y1[kc][h // 2][:, h % 2, :], o2, f"l2_{_u}",
            )
            if fm_last is not None:
                add_dep_helper(l2_first.ins, fm_last.ins, reason="L2 after mid fillers")
            # y = relu(s2*out2 + t2 + x): DVE affine TSP (194), then the
            # residual add on DVE TT (327) or Pool TT (1111, PATH_C), then
            # relu on ACT (612) or DVE fused (add,max) TSP (194, PATH_A).
            # Stores leave per (oc, half) as soon as the half is applied.
            PATH_C = {(0, 1), (1, 1), (2, 1), (3, 1), (0, 2), (1, 2)}
            PATH_A = {(2, 2), (3, 2), (0, 3), (1, 3), (2, 3), (3, 3)}
            outbigs = [
                persist.tile(
                    [P, HPAIRS, 2, NN], BF16, tag=f"obig_{oc}", name=f"obig_{oc}"
                )
                for oc in range(OC)
            ]

            def apply_tile(oc, hp, s2, t2):
                outbig = outbigs[oc]
                s2c = s2[:, oc : oc + 1]
                t2c = t2[:, oc : oc + 1]
                x_in = xs[hp][oc // 2][:, oc % 2, :, :]
                aff = spool.tile([P, 2, NN], BF16, tag="f1", name="f1", bufs=8)
                nc.vector.tensor_scalar(
                    out=aff, in0=o2[oc][hp], scalar1=s2c, scalar2=t2c,
                    op0=mult, op1=add,
                )
                v = spool.tile([P, 2, NN], BF16, tag="f2", name="f2", bufs=8)
                e_add = nc.gpsimd if (oc, hp) in PATH_C else nc.vector
                e_add.tensor_tensor(out=v, in0=aff, in1=x_in, op=add)
                if (oc, hp) in PATH_A:
                    nc.vector.tensor_scalar(
                        out=outbig[:, hp, :, :], in0=v, scalar1=zero_ap,
                        scalar2=None, op0=amax,
                    )
                else:
                    nc.scalar.activation(
                        out=outbig[:, hp, :, :], in_=v, func=AF.Relu
                    )

            half = 2 * NN

            def apply_oc(oc, s2, t2):
                for hf in range(2):
                    apply_tile(oc, 2 * hf, s2, t2)
                    apply_tile(oc, 2 * hf + 1, s2, t2)
                    nc.sync.dma_start(
                        out=out_d.ap()[_u, oc][:, 2 * hf * half : (2 * hf + 2) * half],
                        in_=outbigs[oc][:, 2 * hf : 2 * hf + 2, :, :].rearrange(
                            "p a b n -> p (a b n)"
                        ),
                    )

            if os.environ.get("KERNEL_APPLY_HOOK", "1") == "1":
                stats_reduce(
                    stats2, gbs["g2"], gbs["b2"], f"l2_{_u}", per_oc_done=apply_oc
                )
            else:
                s2, t2 = stats_reduce(stats2, gbs["g2"], gbs["b2"], f"l2_{_u}")
                for hf in range(2):
                    for oc in range(OC):
                        apply_tile(oc, 2 * hf, s2, t2)
                        apply_tile(oc, 2 * hf + 1, s2, t2)
                        nc.sync.dma_start(
                            out=out_d.ap()[_u, oc][
                                :, 2 * hf * half : (2 * hf + 2) * half
                            ],
                            in_=outbigs[oc][:, 2 * hf : 2 * hf + 2, :, :].rearrange(
                                "p a b n -> p (a b n)"
                            ),
                        )

        nc.scalar.dma_start(out=junk_d.ap()[:, 0:1], in_=dummy_out)
        if NF1 + NFM > 0:
            jnk2 = persist.tile([P, 1], F32, tag="jnk2", name="jnk2")
            nc.scalar.activation(out=jnk2, in_=fpt[:, 0:1], func=AF.Copy)
            nc.scalar.dma_start(out=junk_d.ap()[:, 1:2], in_=jnk2)

    nc.compile()
    return nc


def _get_program():
    global _cached
    if _cached is None:
        _cached = _build_program()
    return _cached


def _pack_inputs(x, w1, g1, b1, w2, g2, b2):
    """Host-side shard + repack into the device layouts (see module doc)."""
    bf16 = ml_dtypes.bfloat16
    # x: (N, C, H) -> (KC, P, H, N) -> per-core (HPAIRS, P, KC, 2, N)
    xt = np.ascontiguousarray(x.transpose(1, 2, 0)).reshape(KC, P, H, N)
    xt = xt.astype(bf16)

    # w: (O, C, H) -> (H, P, KC, C)
    def packw(w):
        wt = w.transpose(2, 1, 0).reshape(H, KC, P, C).transpose(0, 2, 1, 3)
        return wt.astype(bf16)

    w1t = packw(w1)
    w2t = packw(w2)

    def packg(v):
        return np.ascontiguousarray(v.reshape(OC, P).T.astype(np.float32))

    gbt = np.empty((P, GB_W), np.float32)
    gbt[:, 0 * OC : 1 * OC] = packg(g1)
    gbt[:, 1 * OC : 2 * OC] = packg(b1)
    gbt[:, 2 * OC : 3 * OC] = packg(g2)
    gbt[:, 3 * OC : 4 * OC] = packg(b2)
    gbt[:, 4 * OC + 0] = EPS
    gbt[:, 4 * OC + 1] = 1.0 / NCORES
    gbt[:, 4 * OC + 2] = 1.0
    gbt[:, 4 * OC + 3] = 0.0
    gbt[:, 4 * OC + 4] = 0.5
    gbt[:, 4 * OC + 5 :] = 0.0

    in_maps = []
    for c in range(NCORES):
        h0, h1 = c * HS, (c + 1) * HS
        # (KC, P, HS, N) -> (KC, P, HPAIRS, 2, N) -> (HPAIRS, P, KC, 2, N)
        xc = xt[:, :, h0:h1, :].reshape(KC, P, HPAIRS, 2, N)
        xc = np.ascontiguousarray(xc.transpose(2, 1, 0, 3, 4))
        in_maps.append(
            {
                "xt": xc,
                "w1t": np.ascontiguousarray(w1t[h0:h1]).reshape(
                    HPAIRS, 2, P, KC, C
                ).transpose(0, 2, 1, 3, 4).copy(),
                "w2t": np.ascontiguousarray(w2t[h0:h1]).reshape(
                    HPAIRS, 2, P, KC, C
                ).transpose(0, 2, 1, 3, 4).copy(),
                "gbt": gbt,
            }
        )
    return in_maps


def kernel(x, w1, g1, b1, w2, g2, b2):
    global LAST_EXEC_NS, LAST_RESULTS
    nc = _get_program()
    in_maps = _pack_inputs(
        np.asarray(x, dtype=np.float32),
        np.asarray(w1, dtype=np.float32),
        np.asarray(g1, dtype=np.float32),
        np.asarray(b1, dtype=np.float32),
        np.asarray(w2, dtype=np.float32),
        np.asarray(g2, dtype=np.float32),
        np.asarray(b2, dtype=np.float32),
    )
    trace = os.environ.get("KERNEL_TRACE", "0") == "1"
    res = run_bass_kernel_spmd(
        nc, in_maps, list(range(NCORES)), trace=trace
    )
    LAST_EXEC_NS = res.exec_time_ns
    LAST_RESULTS = res
    parts = []
    for c in range(NCORES):
        r = np.asarray(res.results[c]["out"])[0]  # [oc, p, (hp hh n)] bf16
        r = r.astype(np.float32).reshape(OC, P, HPAIRS, 2, N)
        # -> (n, c, h): c = oc*P + p, h = 2*hp + hh
        r = r.transpose(4, 0, 1, 2, 3).reshape(N, C, HS)
        parts.append(r)
    return np.concatenate(parts, axis=2).astype(np.float32)


if __name__ == "__main__":
    # smoke test with random data
    rng = np.random.default_rng(0)
    x = rng.standard_normal((N, C, H), dtype=np.float32)
    w1 = rng.standard_normal((C, C, H), dtype=np.float32) * 0.02
    w2 = rng.standard_normal((C, C, H), dtype=np.float32) * 0.02
    g1 = np.ones(C, np.float32)
    b1 = np.zeros(C, np.float32)
    g2 = np.ones(C, np.float32)
    b2 = np.zeros(C, np.float32)
    y = kernel(x=x, w1=w1, g1=g1, b1=b1, w2=w2, g2=g2, b2=b2)
    print(y.shape, y.dtype, float(np.abs(y).max()))


# revision 96
# speedup vs baseline: 1.0255x; 1.0011x over previous
"""Trainium2 Bass kernel for nn_BasicBlock1D (locally-connected 1x1 conv x2
with training-mode BatchNorm, residual, ReLU).

Reference computation (per spatial position h, there are H=64 of them):
    out1[n,o,h] = sum_c x[n,c,h] * w1[o,c,h]          (512x512 matmul per h)
    y1 = relu(bn1(out1))                              (stats over (N,H))
    out2[n,o,h] = sum_c y1[n,c,h] * w2[o,c,h]
    y  = relu(bn2(out2) + x)

Sharding: the 64 spatial positions are split across the 8 NeuronCores (8 per
core).  Each core reads only its h-slice of x/w1/w2, so every HBM byte is
read exactly once chip-wide.  BatchNorm statistics span the full (N,H) batch;
each core pre-aggregates its local per-channel (mean, E[x^2]) and a single
4KB AllReduce(add) with a Shared output buffer combines them:
    mean_g = sum_c mean_c / 8,  var_g = sum_c (var_c + mean_c^2)/8 - mean_g^2
(equal-sized groups of 2048 samples per core make this exact).

Schedule (cost-model-driven):
  The kernel is DMA-floor bound: 10MB of inputs + 2MB of output per core at
  the 360 B/ns aggregate DMA bandwidth is ~33us, against ~27us of bf16 PE
  time; the critical path is L1(DMA 6MB) -> bn1 chain -> L2(PE 13.7us) ->
  bn2 chain -> apply/store tail.

  * All inputs stream on the SP HWDGE ring in exact PE-consumption order at
    256KB granularity (x tiles in 2 halves, each 1MB weight tile in 4
    pieces), so layer 1 starts after ~1.1MB is buffered (~4.5us) and runs
    DMA-paced with the PE never stalling (a stalled PE drops out of its
    boosted clock: 394/213/107ns per 256-row matmul cold/mid/boosted, and
    any idle gap resets the boost).
  * g/b/eps constants ride as ONE batched DMA on the otherwise-idle ACT
    ring at t=0 (each DMA costs 625ns of the globally-serialized HWDGE
    sequencer regardless of size, so five separate loads would stall the
    main stream or arrive after the bn1 chain needs them).
  * PE warm-up fillers (garbage matmuls accumulating into a reserved PSUM
    bank) cover the initial prefetch window, and a second filler block is
    emitted contiguously behind layer 1's last matmul so the PE stays in
    its boosted clock state through the bn1-chain gap and layer 2 starts
    back-to-back at full speed.
  * Structure per h-pair: each PSUM tile is a full bank [128, 2, 256]; 8
    matmuls accumulate into it, emitted hh-MAJOR across the chunk's four
    oc tiles so the PE consumes weight pieces in arrival order instead of
    the first oc-group stalling on the chunk's last piece; a single ACT
    op evacuates each tile to bf16 (walrus rejects GPSIMD touching PSUM
    at all).  bn_stats reads the EVACUATED bf16 tile so the PSUM bank
    frees on the evac alone and the pool rotation (bufs=7) never stalls
    a later chunk's matmuls behind lagging DVE stats -- except the LAST
    h-pair, whose stats read PSUM directly (no evac latency on the
    chain-critical path) as two half-bank tuples so the first half's
    stats overlap the second half's matmuls (bn_aggr combines tuples of
    unequal counts exactly).
  * The stats->coefficient chains are emitted PER output-channel chunk in
    the no-collective build (aggr -> (var+eps)^0.5 via a fused pow
    tensor_scalar -> g/std via tensor_tensor divide -> t), all on DVE so
    there are zero cross-engine handoffs; s[oc]/t[oc] unlock downstream
    per-oc work as soon as that oc's raw stats land.  The collective
    build keeps the proven monolithic Sqrt/reciprocal chain around the
    4KB AllReduce.
  * The final phase per tile: DVE affine (s2*o2 + t2, one fused 2-scalar
    tensor_scalar), residual add on DVE TT or Pool TT (walrus accepts
    ONLY the TensorTensor class on Pool -- no tensor_scalar, no max op),
    then relu as ACT Relu or a fused DVE (add 0, max) tensor_scalar for
    the end-tiles where ACT is congested.  Each output (oc, h-half)
    leaves in a 256KB store the moment its two tiles are applied.

A dummy Sqrt at t=0 preloads the ACT function table off the critical
path.

KERNEL_UNROLL=k (default 1) builds the whole pipeline k times back-to-back
into one NEFF -- a measurement aid.

Stack quirks this kernel deliberately avoids (verified empirically on this
axon/PJRT toolchain): tensor_tensor_reduce (faults), tensor_tensor with the
same tile as both operands, DVE memset feeding scalar operands, float
immediates in tensor_scalar, in-place elementwise ops, instructions whose
only output has no reader (walrus drops the alloc and the engine faults),
any non-TensorTensor compute op on Pool/GPSIMD and any Pool access to PSUM
(both fail walrus's per-engine ISA check), and SBUF tiles written by more
than one DMA (the framework under-gates readers of multi-DMA tiles, so
every DMA gets its own tile).
"""

import os
import sys
from contextlib import ExitStack

import numpy as np

_REPO = "/opt/trn_rl_repo"
if _REPO not in sys.path:
    sys.path.insert(0, _REPO)

import ml_dtypes  # noqa: E402

import concourse.bacc as bacc  # noqa: E402
import concourse.tile as tile  # noqa: E402
from concourse import mybir  # noqa: E402
from concourse.bass_utils import run_bass_kernel_spmd  # noqa: E402

N, C, H = 256, 512, 64
NCORES = 8
HS = H // NCORES  # 8 h positions per core
P = 128
KC = C // P  # 4 contraction chunks
OC = C // P  # 4 output-channel chunks
NN = N  # moving free dim of each matmul
HPAIRS = HS // 2  # weight tiles / activation tiles hold 2 h positions
EPS = 1e-5
GB_W = 4 * OC + 8  # packed g/b/cst width

BF16 = mybir.dt.bfloat16
F32 = mybir.dt.float32

LAST_EXEC_NS = None
LAST_RESULTS = None

_cached = None


def _build_program():
    nc = bacc.Bacc(
        "TRN2",
        target_bir_lowering=False,
        debug=False,
        num_devices=NCORES,
    )

    xt_d = nc.dram_tensor("xt", [HPAIRS, P, KC, 2, NN], BF16, kind="ExternalInput")
    w1_d = nc.dram_tensor("w1t", [HPAIRS, P, 2, KC, C], BF16, kind="ExternalInput")
    w2_d = nc.dram_tensor("w2t", [HPAIRS, P, 2, KC, C], BF16, kind="ExternalInput")
    gb_d = nc.dram_tensor("gbt", [P, GB_W], F32, kind="ExternalInput")
    unroll = int(os.environ.get("KERNEL_UNROLL", "1"))
    out_d = nc.dram_tensor(
        "out", [unroll, OC, P, HPAIRS * 2 * NN], BF16, kind="ExternalOutput"
    )
    junk_d = nc.dram_tensor("junk", [P, 2], F32, kind="ExternalOutput")

    add = mybir.AluOpType.add
    mult = mybir.AluOpType.mult
    amax = mybir.AluOpType.max
    AF = mybir.ActivationFunctionType
    use_cc = os.environ.get("KERNEL_NOCC", "0") != "1"

    NF1 = int(os.environ.get("KERNEL_NFILL1", "16"))
    NFM = int(os.environ.get("KERNEL_NFILLM", "16"))

    with tile.TileContext(nc) as tc, ExitStack() as ctx:
        persist = ctx.enter_context(tc.tile_pool(name="persist", bufs=1))
        wpool = ctx.enter_context(tc.tile_pool(name="wpool", bufs=32))
        spool = ctx.enter_context(tc.tile_pool(name="spool", bufs=2))
        psum = ctx.enter_context(tc.tile_pool(name="psum", bufs=7, space="PSUM"))
        psumf = ctx.enter_context(tc.tile_pool(name="psumf", bufs=1, space="PSUM"))
        fpool = ctx.enter_context(tc.tile_pool(name="fpool", bufs=1))
        dram = ctx.enter_context(tc.tile_pool(name="dram", bufs=1, space="DRAM"))

        def hp_tiles(nm, dt):
            return [
                [
                    persist.tile([P, 2, NN], dt, tag=f"{nm}_{k}_{hp}", name=f"{nm}_{k}_{hp}")
                    for hp in range(HPAIRS)
                ]
                for k in range(OC)
            ]

        # --- persistent activations (x in two kc-half tiles per h-pair so
        # the stream gates the first matmuls at 256KB granularity) ---
        xs = [
            [
                persist.tile([P, 2, 2, NN], BF16, tag=f"x{hp}_{kh}", name=f"x{hp}_{kh}")
                for kh in range(2)
            ]
            for hp in range(HPAIRS)
        ]

        def x_at(hp, kc, hh):
            return xs[hp][kc // 2][:, kc % 2, hh, :]
        raw1 = hp_tiles("r1", BF16)   # layer-1 pre-BN output
        y1 = hp_tiles("y1", BF16)
        o2 = hp_tiles("o2", BF16)     # layer-2 pre-BN output

        # g/b/cst: ONE batched DMA on the ACT ring at t=0 (see module doc).
        gbt = persist.tile([P, GB_W], F32, tag="gbt", name="gbt")
        nc.scalar.dma_start(out=gbt, in_=gb_d.ap())
        gbs = {
            "g1": gbt[:, 0 * OC : 1 * OC],
            "b1": gbt[:, 1 * OC : 2 * OC],
            "g2": gbt[:, 2 * OC : 3 * OC],
            "b2": gbt[:, 3 * OC : 4 * OC],
        }
        eps_ap = gbt[:, 4 * OC + 0 : 4 * OC + 1]
        inv8_ap = gbt[:, 4 * OC + 1 : 4 * OC + 2]  # 1/NCORES
        one_ap = gbt[:, 4 * OC + 2 : 4 * OC + 3]   # 1.0
        zero_ap = gbt[:, 4 * OC + 3 : 4 * OC + 4]  # 0.0
        half_ap = gbt[:, 4 * OC + 4 : 4 * OC + 5]  # 0.5

        # ACT function-table preload: a dummy Sqrt as the very first ACT op
        # pulls in the (sqrt + copy/relu/square) table off the critical path.
        # Its junk store is deferred to program end so the ACT SEQ doesn't
        # sit blocked on the HWDGE ring (busy with the input stream).
        dummy_in = persist.tile([P, 1], F32, tag="dmy_i", name="dmy_i")
        nc.vector.memset(dummy_in, 4.0)
        dummy_out = persist.tile([P, 1], F32, tag="dmy_o", name="dmy_o")
        nc.scalar.activation(out=dummy_out, in_=dummy_in, func=AF.Sqrt)

        # --- PE warm-up fillers (see module doc) ---
        filler_w = fpool.tile([P, P], BF16, tag="fil_w", name="fil_w")
        nc.gpsimd.memset(filler_w, 0.0)
        filler_x = fpool.tile([P, NN], BF16, tag="fil_x", name="fil_x")
        nc.gpsimd.memset(filler_x, 0.0)
        fpt = psumf.tile([P, NN], F32, tag="fil_p", name="fil_p")


        def pe_fill(n):
            # self-contained accumulation group per block: a group that
            # spans other groups' matmuls trips the scheduler into
            # serializing it behind unrelated work.  Returns (first, last)
            # instruction handles so callers can pin the block's position
            # in the PE stream with explicit deps (the tile scheduler
            # otherwise hoists dep-free matmuls into any earlier PE idle
            # slot).
            first = last = None
            for i in range(n):
                h = nc.tensor.matmul(
                    fpt, lhsT=filler_w, rhs=filler_x,
                    start=(i == 0), stop=(i == n - 1),
                )
                if first is None:
                    first = h
                last = h
            return first, last

        def w_piece(w_d, hp, hh, kcp):
            # a 256KB piece-TILE of a weight tile, in PE-consumption order:
            # (hh0,kc01) (hh0,kc23) (hh1,kc01) (hh1,kc23).  One DMA per
            # tile: the framework's DMA-completion waits under-gate tiles
            # written by multiple DMAs (verified in the timeline sim), so a
            # piece must be its own tile for readers to be sequenced right.
            wp = wpool.tile([P, 2, C], BF16, tag="wp", name="wp")
            nc.sync.dma_start(
                out=wp, in_=w_d.ap()[hp][:, hh, 2 * kcp : 2 * kcp + 2, :]
            )
            return wp

        def stream_l1(hp):
            # x halves interleaved with w pieces in first-need order
            nc.sync.dma_start(out=xs[hp][0], in_=xt_d.ap()[hp][:, 0:2])
            p00 = w_piece(w1_d, hp, 0, 0)
            p01 = w_piece(w1_d, hp, 0, 1)
            nc.sync.dma_start(out=xs[hp][1], in_=xt_d.ap()[hp][:, 2:4])
            p10 = w_piece(w1_d, hp, 1, 0)
            p11 = w_piece(w1_d, hp, 1, 1)
            return [p00, p01, p10, p11]

        def stream_w2(hp):
            return [w_piece(w2_d, hp, hh, kcp) for hh in range(2) for kcp in range(2)]

        def layer(w_tiles, src_at, dst_tiles, lname, tail_pairs=False):
            """Per-position matmuls + per-(chunk,h-pair) raw BN moments.

            src_at(kc, h) -> [P, NN] AP of the layer input
            dst_tiles[oc][hp][:, hh, :] <- the (h = 2*hp+hh) output slice
            tail_pairs: process the last two h-pairs interleaved by
            oc-PAIRS ((hp2,oc01),(hp3,oc01),(hp2,oc23),(hp3,oc23)) so
            oc0/oc1's raw stats complete ~2.6us before the layer ends and
            their coefficient chains + downstream applies/stores overlap
            the remaining matmuls.
            returns (stats tile [P, OC, HPAIRS, 6], first mm, last mm).
            """
            # HPAIRS+1 raw-stat slots per oc: one PSUM-direct tuple per
            # h-pair for hp<3, and TWO half-bank PSUM tuples for the last
            # h-pair so its first half's stats run while the second half's
            # matmuls are still going (bn_aggr combines tuples of unequal
            # counts exactly).
            st_raw = persist.tile(
                [P, OC, HPAIRS + 1, 6], F32, tag=f"straw_{lname}", name=f"straw_{lname}"
            )
            order = [(hp, oc) for hp in range(HPAIRS) for oc in range(OC)]
            if tail_pairs:
                order = [(hp, oc) for hp in (0, 1) for oc in range(OC)] + [
                    (hp, oc)
                    for ocp in (0, 1)
                    for hp in (2, 3)
                    for oc in (2 * ocp, 2 * ocp + 1)
                ]
            first_mm = last_mm = None
            for hp in range(HPAIRS):
                pieces = w_tiles[hp]
                # hh-MAJOR matmul emission within the chunk: the first 16
                # matmuls need only the (hh0) weight pieces, so the PE
                # consumes the stream as it arrives instead of the first
                # oc-group stalling on the chunk's last piece.  Each
                # pt[:, hh, :] slice is its own accumulation group either
                # way.  For the last h-pair this also completes all hh0
                # half-bank stats inputs 1.7us before the layer ends.
                pts = [None] * OC
                for hh in range(2):
                    h = hp * 2 + hh
                    for oc in range(OC):
                        if pts[oc] is None:
                            # allocate at first use so the pool-rotation
                            # waits stagger instead of bunching at the
                            # chunk boundary
                            pts[oc] = psum.tile([P, 2, NN], F32, tag="ps", name="ps")
                        for kc in range(KC):
                            wp = pieces[2 * hh + kc // 2]
                            m = nc.tensor.matmul(
                                pts[oc][:, hh, :],
                                lhsT=wp[:, kc % 2, oc * P : (oc + 1) * P],
                                rhs=src_at(kc, h),
                                start=(kc == 0),
                                stop=(kc == KC - 1),
                            )
                            if first_mm is None:
                                first_mm = m
                            last_mm = m
                    if hp == HPAIRS - 1:
                        # half-bank stats as soon as this hh's groups close
                        for oc in range(OC):
                            nc.vector.bn_stats(
                                out=st_raw[:, oc, hp + hh, :],
                                in_=pts[oc][:, hh, :],
                            )
                for oc in range(OC):
                    # single-op PSUM evacuation on ACT (the only engine
                    # that can both read PSUM and run concurrently here:
                    # GPSIMD cannot access PSUM at all); bn_stats reads the
                    # PSUM bank directly so it runs concurrently with the
                    # evacuation instead of after it
                    nc.scalar.activation(
                        out=dst_tiles[oc][hp], in_=pts[oc], func=AF.Copy
                    )
                    if hp < HPAIRS - 1:
                        # read the evacuated bf16 tile, not PSUM: the bank
                        # then frees on the ACT evac alone, so the pool
                        # rotation (bufs=7) never stalls a later chunk's
                        # matmuls behind this tile's (lagging) DVE stats
                        nc.vector.bn_stats(
                            out=st_raw[:, oc, hp, :],
                            in_=dst_tiles[oc][hp].rearrange("p a n -> p (a n)"),
                        )
            return st_raw, first_mm, last_mm

        def stats_reduce(st_raw, g_t, b_t, lname, per_oc_done=None):
            """Local aggregate -> 4KB AllReduce(add) -> BN scale/shift.

            bn(v) = s*v + t with s = g/sqrt(var+eps), t = b - mean*s.
            """

            def small(nm, shape=(P, OC)):
                return persist.tile(
                    list(shape), F32, tag=f"{nm}_{lname}", name=f"{nm}_{lname}"
                )

            # Everything below stays on DVE (ACT only for the one Sqrt) --
            # each cross-engine handoff in this serial chain costs ~300ns.
            mv = small("mv", (P, OC, 2))
            if not use_cc:
                # per-oc chains: s[oc]/t[oc] unlock as soon as THAT oc's
                # last raw stats land, so downstream per-oc consumers
                # (y1 h-pair 0, the final apply) start ~1-2us earlier than
                # a monolithic chain would allow.
                std = small("std")
                s_t = small("s")
                mts = small("mts")
                t_t = small("t")
                for oc in range(OC):
                    sl = slice(oc, oc + 1)
                    nc.vector.bn_aggr(out=mv[:, oc, :], in_=st_raw[:, oc, :, :])
                    nc.vector.tensor_scalar(
                        out=std[:, sl], in0=mv[:, oc, 1:2], scalar1=eps_ap,
                        scalar2=half_ap, op0=add, op1=mybir.AluOpType.pow,
                    )
                    nc.vector.tensor_tensor(
                        out=s_t[:, sl], in0=g_t[:, sl], in1=std[:, sl],
                        op=mybir.AluOpType.divide,
                    )
                    nc.vector.tensor_tensor(
                        out=mts[:, sl], in0=mv[:, oc, 0:1], in1=s_t[:, sl],
                        op=mult,
                    )
                    nc.vector.tensor_tensor(
                        out=t_t[:, sl], in0=b_t[:, sl], in1=mts[:, sl],
                        op=mybir.AluOpType.subtract,
                    )
                    if per_oc_done is not None:
                        per_oc_done(oc, s_t, t_t)
                return s_t, t_t
            for oc in range(OC):
                nc.vector.bn_aggr(out=mv[:, oc, :], in_=st_raw[:, oc, :, :])
            if True:
                # pack local (mean, var + mean^2) pairs
                cin = small("cin", (P, OC, 2))
                nc.vector.tensor_scalar(
                    out=cin[:, :, 0], in0=mv[:, :, 0], scalar1=one_ap, scalar2=None,
                    op0=mult,
                )
                msq = small("msq")
                nc.vector.tensor_mul(out=msq, in0=mv[:, :, 0], in1=cin[:, :, 0])
                nc.vector.tensor_add(out=cin[:, :, 1], in0=mv[:, :, 1], in1=msq)
                cci = dram.tile([P, 2 * OC], F32, tag=f"cci_{lname}", name=f"cci_{lname}")
                cco = dram.tile(
                    [P, 2 * OC], F32, tag=f"cco_{lname}", name=f"cco_{lname}",
                    addr_space="Shared",
                )
                nc.scalar.dma_start(out=cci, in_=cin.rearrange("p a b -> p (a b)"))
                nc.gpsimd.collective_compute(
                    "AllReduce",
                    add,
                    replica_groups=[list(range(NCORES))],
                    ins=[cci.opt()],
                    outs=[cco.opt()],
                )
                red = small("red", (P, OC, 2))
                nc.scalar.dma_start(
                    out=red, in_=cco.rearrange("p (a b) -> p a b", a=OC)
                )
                me2 = small("me2", (P, OC, 2))
                nc.vector.tensor_scalar(
                    out=me2, in0=red, scalar1=inv8_ap, scalar2=None, op0=mult
                )
                m_g = me2[:, :, 0]
                mr = small("mr")
                nc.vector.tensor_mul(out=mr, in0=m_g, in1=red[:, :, 0])  # m^2 * 8
                varg = small("varg")  # var * 8
                nc.vector.tensor_sub(out=varg, in0=red[:, :, 1], in1=mr)
                std = small("std")
                nc.scalar.activation(
                    out=std, in_=varg, func=AF.Sqrt, bias=eps_ap, scale=inv8_ap
                )
                rstd = small("rstd")
                nc.vector.reciprocal(out=rstd, in_=std)
                s_t = small("s")
                nc.vector.tensor_mul(out=s_t, in0=rstd, in1=g_t)
            mts = small("mts")
            nc.vector.tensor_mul(out=mts, in0=m_g, in1=s_t)
            t_t = small("t")
            nc.vector.tensor_sub(out=t_t, in0=b_t, in1=mts)
            if per_oc_done is not None:
                for oc in range(OC):
                    per_oc_done(oc, s_t, t_t)
            return s_t, t_t

        from concourse.tile_rust import add_dep_helper

        for _u in range(unroll):
            _, f1_last = pe_fill(NF1)
            # ---- input stream: exact PE-consumption order on the SP ring
            w1_tiles = [stream_l1(hp) for hp in range(HPAIRS)]
            w2_tiles = [stream_w2(hp) for hp in range(HPAIRS)]

            # ---------------- layer 1 ----------------
            stats1, l1_first, l1_last = layer(
                w1_tiles, lambda kc, h: x_at(h // 2, kc, h % 2), raw1, f"l1_{_u}",
            )
            # keep the PE boosted through the bn1-chain gap: fillers run
            # back-to-back behind layer 1's last matmul, and layer 2's
            # first matmul queues back-to-back behind the fillers.  The
            # tile scheduler hoists dep-free matmuls into any earlier PE
            # idle slot, so the block is pinned with explicit deps.
            if f1_last is not None:
                add_dep_helper(l1_first.ins, f1_last.ins, reason="NF1 before L1")
            fm_first, fm_last = pe_fill(NFM)
            if fm_first is not None:
                add_dep_helper(fm_first.ins, l1_last.ins, reason="mid fillers after L1")
            # y1 = relu(s1*out1 + t1).  The h-pair-0 tile for each oc is
            # emitted from the per-oc chain hook so the first layer-2
            # matmul group unblocks as early as possible.  Engine split
            # balances hp0 latency (ACT 1-op + DVE 2-op) and mid-L2
            # occupancy (ACT/DVE/Pool); Pool tiles go on late h-pairs
            # since its 2-op path is ~3.2us for 2 tiles.
            Y1_ACT = {(2, 0), (3, 0), (0, 1), (1, 1), (0, 2), (1, 2), (0, 3)}

            def y1_tile(oc, hp, s1, t1):
                if (oc, hp) in Y1_ACT:
                    nc.scalar.activation(
                        out=y1[oc][hp],
                        in_=raw1[oc][hp],
                        func=AF.Relu,
                        scale=s1[:, oc : oc + 1],
                        bias=t1[:, oc : oc + 1],
                    )
                else:
                    ytmp = spool.tile([P, 2, NN], BF16, tag="ya", name="ya", bufs=3)
                    nc.vector.tensor_scalar(
                        out=ytmp,
                        in0=raw1[oc][hp],
                        scalar1=s1[:, oc : oc + 1],
                        scalar2=t1[:, oc : oc + 1],
                        op0=mult,
                        op1=add,
                    )
                    nc.vector.tensor_scalar(
                        out=y1[oc][hp], in0=ytmp, scalar1=zero_ap,
                        scalar2=None, op0=amax,
                    )

            hook1 = (
                (lambda oc, s, t: y1_tile(oc, 0, s, t))
                if os.environ.get("KERNEL_Y1_HOOK", "0") == "1"
                else None
            )
            s1, t1 = stats_reduce(
                stats1, gbs["g1"], gbs["b1"], f"l1_{_u}", per_oc_done=hook1
            )
            for hp in range(0 if hook1 is None else 1, HPAIRS):
                for oc in range(OC):
                    y1_tile(oc, hp, s1, t1)

            # ---------------- layer 2 ----------------
            stats2, l2_first, l2_last = layer(
                w2_tiles, lambda kc, h: y1[kc][h // 2][:, h % 2, :], o2, f"l2_{_u}",
            )
            if fm_last is not None:
                add_dep_helper(l2_first.ins, fm_last.ins, reason="L2 after mid fillers")
            # y = relu(s2*out2 + t2 + x): DVE affine TSP (194), then the
            # residual add on DVE TT (327) or Pool TT (1111, PATH_C), then
            # relu on ACT (612) or DVE fused (add,max) TSP (194, PATH_A).
            # Stores leave per (oc, half) as soon as the half is applied.
            PATH_C = {(0, 1), (1, 1), (2, 1), (3, 1), (0, 2), (1, 2)}
            PATH_A = {(2, 2), (3, 2), (0, 3), (1, 3), (2, 3), (3, 3)}
            outbigs = [
                persist.tile(
                    [P, HPAIRS, 2, NN], BF16, tag=f"obig_{oc}", name=f"obig_{oc}"
                )
                for oc in range(OC)
            ]

            def apply_tile(oc, hp, s2, t2):
                outbig = outbigs[oc]
                s2c = s2[:, oc : oc + 1]
                t2c = t2[:, oc : oc + 1]
                x_in = xs[hp][oc // 2][:, oc % 2, :, :]
                aff = spool.tile([P, 2, NN], BF16, tag="f1", name="f1", bufs=8)
                nc.vector.tensor_scalar(
                    out=aff, in0=o2[oc][hp], scalar1=s2c, scalar2=t2c,
                    op0=mult, op1=add,
                )
                v = spool.tile([P, 2, NN], BF16, tag="f2", name="f2", bufs=8)
                e_add = nc.gpsimd if (oc, hp) in PATH_C else nc.vector
                e_add.tensor_tensor(out=v, in0=aff, in1=x_in, op=add)
                if (oc, hp) in PATH_A:
                    nc.vector.tensor_scalar(
                        out=outbig[:, hp, :, :], in0=v, scalar1=zero_ap,
                        scalar2=None, op0=amax,
                    )
                else:
                    nc.scalar.activation(
                        out=outbig[:, hp, :, :], in_=v, func=AF.Relu
                    )

            half = 2 * NN

            def apply_oc(oc, s2, t2):
                for hf in range(2):
                    apply_tile(oc, 2 * hf, s2, t2)
                    apply_tile(oc, 2 * hf + 1, s2, t2)
                    if oc == OC - 1 and hf == 1:
                        # the very last store sits wholly on the critical
                        # end: split it so the hp2 half leaves early and
                        # only a 128KB transfer follows the last apply
                        for hpp in (2, 3):
                            nc.sync.dma_start(
                                out=out_d.ap()[_u, oc][
                                    :, hpp * half : (hpp + 1) * half
                                ],
                                in_=outbigs[oc][:, hpp, :, :].rearrange(
                                    "p b n -> p (b n)"
                                ),
                            )
                    else:
                        nc.sync.dma_start(
                            out=out_d.ap()[_u, oc][
                                :, 2 * hf * half : (2 * hf + 2) * half
                            ],
                            in_=outbigs[oc][:, 2 * hf : 2 * hf + 2, :, :].rearrange(
                                "p a b n -> p (a b n)"
                            ),
                        )

            if os.environ.get("KERNEL_APPLY_HOOK", "1") == "1":
                stats_reduce(
                    stats2, gbs["g2"], gbs["b2"], f"l2_{_u}", per_oc_done=apply_oc
                )
            else:
                s2, t2 = stats_reduce(stats2, gbs["g2"], gbs["b2"], f"l2_{_u}")
                for hf in range(2):
                    for oc in range(OC):
                        apply_tile(oc, 2 * hf, s2, t2)
                        apply_tile(oc, 2 * hf + 1, s2, t2)
                        nc.sync.dma_start(
                            out=out_d.ap()[_u, oc][
                                :, 2 * hf * half : (2 * hf + 2) * half
                            ],
                            in_=outbigs[oc][:, 2 * hf : 2 * hf + 2, :, :].rearrange(
                                "p a b n -> p (a b n)"
                            ),
                        )

        nc.scalar.dma_start(out=junk_d.ap()[:, 0:1], in_=dummy_out)
        if NF1 + NFM > 0:
            jnk2 = persist.tile([P, 1], F32, tag="jnk2", name="jnk2")
            nc.scalar.activation(out=jnk2, in_=fpt[:, 0:1], func=AF.Copy)
            nc.scalar.dma_start(out=junk_d.ap()[:, 1:2], in_=jnk2)

    nc.compile()
    return nc


def _get_program():
    global _cached
    if _cached is None:
        _cached = _build_program()
    return _cached


def _pack_inputs(x, w1, g1, b1, w2, g2, b2):
    """Host-side shard + repack into the device layouts (see module doc)."""
    bf16 = ml_dtypes.bfloat16
    # x: (N, C, H) -> (KC, P, H, N) -> per-core (HPAIRS, P, KC, 2, N)
    xt = np.ascontiguousarray(x.transpose(1, 2, 0)).reshape(KC, P, H, N)
    xt = xt.astype(bf16)

    # w: (O, C, H) -> (H, P, KC, C)
    def packw(w):
        wt = w.transpose(2, 1, 0).reshape(H, KC, P, C).transpose(0, 2, 1, 3)
        return wt.astype(bf16)

    w1t = packw(w1)
    w2t = packw(w2)

    def packg(v):
        return np.ascontiguousarray(v.reshape(OC, P).T.astype(np.float32))

    gbt = np.empty((P, GB_W), np.float32)
    gbt[:, 0 * OC : 1 * OC] = packg(g1)
    gbt[:, 1 * OC : 2 * OC] = packg(b1)
    gbt[:, 2 * OC : 3 * OC] = packg(g2)
    gbt[:, 3 * OC : 4 * OC] = packg(b2)
    gbt[:, 4 * OC + 0] = EPS
    gbt[:, 4 * OC + 1] = 1.0 / NCORES
    gbt[:, 4 * OC + 2] = 1.0
    gbt[:, 4 * OC + 3] = 0.0
    gbt[:, 4 * OC + 4] = 0.5
    gbt[:, 4 * OC + 5 :] = 0.0

    in_maps = []
    for c in range(NCORES):
        h0, h1 = c * HS, (c + 1) * HS
        # (KC, P, HS, N) -> (KC, P, HPAIRS, 2, N) -> (HPAIRS, P, KC, 2, N)
        xc = xt[:, :, h0:h1, :].reshape(KC, P, HPAIRS, 2, N)
        xc = np.ascontiguousarray(xc.transpose(2, 1, 0, 3, 4))
        in_maps.append(
            {
                "xt": xc,
                "w1t": np.ascontiguousarray(w1t[h0:h1]).reshape(
                    HPAIRS, 2, P, KC, C
                ).transpose(0, 2, 1, 3, 4).copy(),
                "w2t": np.ascontiguousarray(w2t[h0:h1]).reshape(
                    HPAIRS, 2, P, KC, C
                ).transpose(0, 2, 1, 3, 4).copy(),
                "gbt": gbt,
            }
        )
    return in_maps


def kernel(x, w1, g1, b1, w2, g2, b2):
    global LAST_EXEC_NS, LAST_RESULTS
    nc = _get_program()
    in_maps = _pack_inputs(
        np.asarray(x, dtype=np.float32),
        np.asarray(w1, dtype=np.float32),
        np.asarray(g1, dtype=np.float32),
        np.asarray(b1, dtype=np.float32),
        np.asarray(w2, dtype=np.float32),
        np.asarray(g2, dtype=np.float32),
        np.asarray(b2, dtype=np.float32),
    )
    trace = os.environ.get("KERNEL_TRACE", "0") == "1"
    res = run_bass_kernel_spmd(
        nc, in_maps, list(range(NCORES)), trace=trace
    )
    LAST_EXEC_NS = res.exec_time_ns
    LAST_RESULTS = res
    parts = []
    for c in range(NCORES):
        r = np.asarray(res.results[c]["out"])[0]  # [oc, p, (hp hh n)] bf16
        r = r.astype(np.float32).reshape(OC, P, HPAIRS, 2, N)
        # -> (n, c, h): c = oc*P + p, h = 2*hp + hh
        r = r.transpose(4, 0, 1, 2, 3).reshape(N, C, HS)
        parts.append(r)
    return np.concatenate(parts, axis=2).astype(np.float32)


if __name__ == "__main__":
    # smoke test with random data
    rng = np.random.default_rng(0)
    x = rng.standard_normal((N, C, H), dtype=np.float32)
    w1 = rng.standard_normal((C, C, H), dtype=np.float32) * 0.02
    w2 = rng.standard_normal((C, C, H), dtype=np.float32) * 0.02
    g1 = np.ones(C, np.float32)
    b1 = np.zeros(C, np.float32)
    g2 = np.ones(C, np.float32)
    b2 = np.zeros(C, np.float32)
    y = kernel(x=x, w1=w1, g1=g1, b1=b1, w2=w2, g2=g2, b2=b2)
    print(y.shape, y.dtype, float(np.abs(y).max()))
